# revision 5
# baseline (speedup 1.0000x reference)
"""Trainium2 Bass kernel for a batched linear-chain CRF negative log-likelihood.

reference semantics (B=128, S=2048, T=128):
    forward algorithm over S steps -> log_Z per batch
    gold path score = emissions gathered at tags + transitions gathered at
    (tag_t, tag_{t+1}) pairs, summed over time
    output = mean(log_Z - seq_score)   (scalar f32)

Strategy (v3): segmented rank-1 forward algorithm.
  - data parallel over 8 cores: 16 batch rows per core, transitions replicated.
  - linear space: M_t = diag(E_t) W^T with W = exp(transitions),
    E_t = exp(emit_t - chat).  Z = 1^T M_{S-1}..M_1 a0,  a0 = E_0.
  - split S into K=16 segments of L=128.  Products of positive matrices
    contract to rank-1, so P_k ~ f_k g_k^T / s_k with f_k = P_k 1,
    g_k = P_k^T 1, s_k = 1^T P_k 1.  Then
      ln Z ~ sum_k ln(g_k . f_{k-1}) - sum_k ln(s_k) + parked renorm logs
    with segment 0 run on the true a0 and segment K-1 only backward.
    (validated vs fp64 reference: |err| < 0.06 absolute on lnZ ~ 12000.)
  - all 15 fw chains and 15 bw chains advance together: ONE [128x480]
    moving-operand matmul pair per rotation (fw cols 0:240 with stationary W,
    bw cols 240:480 with stationary W^T) into one PSUM bank, then ONE DVE
    multiply with a host-prelaid E slice (rotation-major layout) updates the
    joint state.  127 rotations instead of 2047 sequential steps.
  - gold path: per (b, sblock) one-hot count matmuls accumulate a single
    C|D PSUM region for the whole core (mean-only output); 2 units per
    rotation interleave into PE/DVE/GPSIMD idle time.
  - host prep: bf16 cast + layout permutations of emissions (E layouts in
    rotation-major order, gold in natural order), tag columns as f32.
"""

import numpy as np
import ml_dtypes

B, S, T = 128, 2048, 128
NCORES = 8
BC = B // NCORES        # 16 batch rows per core
L = 128                 # segment length (rotations)
K = S // L              # 16 segments
NCH = K - 1             # 15 chains per direction
NW = NCH * BC           # 240 state columns per direction
NSB = S // 128          # 16 s-blocks for gold
NUNITS = BC * NSB       # 256 gold units
JUNK_TAG = 60000.0
RENORMS = (42, 84, 126)
ECH = 4                 # E chunk: rotations per DMA (4 * 960 cols)

_compiled = None


def _build_program(use_gpsimd=True):
    import concourse.bass as bass
    import concourse.bacc as bacc
    import concourse.tile as tile
    from concourse import mybir
    from concourse.masks import make_identity

    fp32 = mybir.dt.float32
    bf16 = mybir.dt.bfloat16
    AF = mybir.ActivationFunctionType
    ALU = mybir.AluOpType
    AX = mybir.AxisListType

    nc = bacc.Bacc(None)
    # E layout: col r*480 + 0:240 = fw slice (k*16+b -> E[b, k*L+r, :]),
    #           col r*480 + 240:480 = bw slice ((k-1)*16+b -> E[b, (k+1)*L-1-r, :])
    e_d = nc.declare_dram_parameter("e_lay", [128, L * 2 * NW], bf16, isOutput=False)
    eg_d = nc.declare_dram_parameter("emis_gold", [BC, S, T], bf16, isOutput=False)
    tr_d = nc.declare_dram_parameter("transitions", [T, T], fp32, isOutput=False)
    tc_d = nc.declare_dram_parameter("tag_cols", [128, NSB * BC], fp32, isOutput=False)
    ts_d = nc.declare_dram_parameter("tagsh_cols", [128, NSB * BC], fp32, isOutput=False)
    out_d = nc.declare_dram_parameter("loss_parts", [1], fp32, isOutput=True)

    W2 = 2 * NW  # 480

    with tile.TileContext(nc) as tc:
        with (
            tc.tile_pool(name="consts", bufs=1) as consts,
            tc.tile_pool(name="ebuf", bufs=1) as ebufp,
            tc.tile_pool(name="estg", bufs=3) as estgp,
            tc.tile_pool(name="state", bufs=3) as statep,
            tc.tile_pool(name="ring", bufs=3) as ringp,
            tc.tile_pool(name="oh", bufs=6) as ohp,
            tc.tile_pool(name="small", bufs=4) as smallp,
            tc.tile_pool(name="dump", bufs=2) as dumpp,
            tc.tile_pool(name="q_ps", bufs=2, space="PSUM") as q_ps,
            tc.tile_pool(name="g_ps", bufs=1, space="PSUM") as g_ps,
            tc.tile_pool(name="m_ps", bufs=2, space="PSUM") as m_ps,
            tc.tile_pool(name="tp_ps", bufs=1, space="PSUM") as tp_ps,
        ):
            # ---------------- constants ----------------
            ident = consts.tile([128, 128], fp32)
            make_identity(nc, ident)
            ident_bf = consts.tile([128, 128], bf16)
            make_identity(nc, ident_bf)
            iota = consts.tile([128, 128], bf16)
            nc.gpsimd.iota(
                iota, pattern=[[1, 128]], base=0, channel_multiplier=0,
                allow_small_or_imprecise_dtypes=True,
            )
            ones_col_bf = consts.tile([128, 1], bf16)
            nc.vector.memset(ones_col_bf, 1.0)
            ones_col_f = consts.tile([128, 1], fp32)
            nc.vector.memset(ones_col_f, 1.0)
            ones_row_f = consts.tile([1, 128], fp32)
            nc.vector.memset(ones_row_f, 1.0)

            tr_sb = consts.tile([128, 128], fp32)
            nc.sync.dma_start(out=tr_sb, in_=tr_d[:, :])
            tag_cols = consts.tile([128, NSB * BC], fp32)
            nc.sync.dma_start(out=tag_cols, in_=tc_d[:, :])
            tagsh_cols = consts.tile([128, NSB * BC], fp32)
            nc.sync.dma_start(out=tagsh_cols, in_=ts_d[:, :])

            w_bf = consts.tile([128, 128], bf16)
            nc.scalar.activation(w_bf, tr_sb, AF.Exp)
            wt_psum = tp_ps.tile([128, 128], bf16, tag="tp")
            nc.tensor.transpose(wt_psum, w_bf, ident_bf)
            wt_bf = consts.tile([128, 128], bf16)
            nc.vector.tensor_copy(wt_bf, wt_psum)

            # [trans | identity] for the gold finalize
            tri = consts.tile([128, 256], fp32)
            nc.vector.tensor_copy(tri[:, 0:128], tr_sb)
            nc.vector.tensor_copy(tri[:, 128:256], ident)

            # rho = W^T 1 (colsums of W) as [128,1];  chat = mean_{j>=1} ln rho_j
            rho_ps = m_ps.tile([128, 1], fp32, tag="m")
            nc.tensor.matmul(rho_ps, w_bf, ones_col_bf, start=True, stop=True)
            rho = consts.tile([128, 1], fp32)
            nc.vector.tensor_copy(rho, rho_ps)
            colw_ps = m_ps.tile([1, 128], fp32, tag="m")
            nc.tensor.matmul(colw_ps, ones_col_bf, w_bf, start=True, stop=True)
            lncol = smallp.tile([1, 127], fp32, tag="lncol")
            lnsum = consts.tile([1, 1], fp32)
            nc.scalar.activation(lncol, colw_ps[:, 1:128], AF.Ln, accum_out=lnsum)
            chat16 = consts.tile([1, 1], fp32)
            # 16 batch rows * S * chat  (chat = lnsum/127)
            nc.scalar.activation(chat16, lnsum, AF.Copy, scale=float(BC) * float(S) / 127.0)
            negchat = consts.tile([1, 1], fp32)
            nc.scalar.activation(negchat, lnsum, AF.Copy, scale=-1.0 / 127.0)
            nbc_ps = m_ps.tile([128, 1], fp32, tag="m")
            nc.tensor.matmul(nbc_ps, ones_row_f, negchat, start=True, stop=True)
            negchat_bc = consts.tile([128, 1], fp32)
            nc.vector.tensor_copy(negchat_bc, nbc_ps)

            # parked renorm logs: bw 3x240 | fw(k=0 only) 3x16
            glog = consts.tile([1, 3 * NW + 3 * BC], fp32)
            nc.vector.memset(glog, 1.0)

            # ---------------- E supply (DMA + exp) ----------------
            ebuf = ebufp.tile([128, L * W2], bf16)  # 128 x 61440, 120KB/part

            def emit_echunk(c):
                # rotations [c*ECH, (c+1)*ECH)
                cols = ECH * W2
                stg = estgp.tile([128, cols], bf16, tag="estg")
                nc.sync.dma_start(out=stg, in_=e_d[:, c * cols:(c + 1) * cols])
                nc.scalar.activation(
                    ebuf[:, c * cols:(c + 1) * cols], stg, AF.Exp, bias=negchat_bc
                )

            emit_echunk(0)
            emit_echunk(1)

            # ---------------- gold machinery ----------------
            gold_ps = g_ps.tile([128, 512], fp32, tag="gold")
            gp = nc.gpsimd if use_gpsimd else nc.vector

            rings = {}     # ring index -> tile (4 units each)
            ohs = {}       # unit -> oh tile

            def gold_dma(ri):
                # units 4ri..4ri+3: same b, sb = 4*(ri%4)..; unit u = b*16+sb
                b = (4 * ri) // NSB
                sb0 = (4 * ri) % NSB
                ring = ringp.tile([128, 1024], bf16, tag="ring", name=f"ring_{ri}")
                # dst[p, u*256+128 : u*256+256] = emis[b, (sb0+u)*128 + p, :]
                src = eg_d[b, sb0 * 128:(sb0 + 4) * 128, :].rearrange(
                    "(u p) t -> p u t", u=4
                )
                dst = ring.rearrange("p (u c) -> p u c", u=4)[:, :, 128:256]
                nc.sync.dma_start(out=dst, in_=src)
                rings[ri] = ring

            def gold_build(u):
                # one-hots for unit u: oh (DVE), ohsh into ring left half (gpsimd)
                b, sb = u // NSB, u % NSB
                col = sb * BC + b
                oh = ohp.tile([128, 128], bf16, tag="oh", name=f"oh_{u}")
                nc.vector.tensor_scalar(
                    out=oh, in0=iota, scalar1=tag_cols[:, col:col + 1],
                    scalar2=None, op0=ALU.is_equal,
                )
                ring = rings[u // 4]
                dst = ring.rearrange("p (u c) -> p u c", u=4)[:, u % 4, 0:128]
                gp.tensor_scalar(
                    out=dst, in0=iota, scalar1=tagsh_cols[:, col:col + 1],
                    scalar2=None, op0=ALU.is_equal,
                )
                ohs[u] = oh

            def gold_mm(u):
                oh = ohs.pop(u)
                ring = rings[u // 4]
                mv = ring.rearrange("p (u c) -> p u c", u=4)[:, u % 4, :]
                nc.tensor.matmul(
                    gold_ps[:, 0:256], oh, mv,
                    start=(u == 0), stop=(u == NUNITS - 1),
                )
                if u % 4 == 3:
                    rings.pop(u // 4)

            # prime gold pipeline: ring 0 + one-hots for units 0, 1
            gold_dma(0)
            gold_build(0)
            gold_build(1)

            # ---------------- chain states ----------------
            # joint state [128 x 480]: fw cols 0:240 (chain k*16+b covers seg k,
            # k=0..14), bw cols 240:480 ((k-1)*16+b covers seg k, k=1..15)
            st = statep.tile([128, W2], bf16, tag="st")
            # fw init: seg0 = E[:,0:16]; interior = E * rho
            nc.vector.tensor_copy(st[:, 0:BC], ebuf[:, 0:BC])
            nc.vector.tensor_scalar(
                out=st[:, BC:NW], in0=ebuf[:, BC:NW], scalar1=rho,
                scalar2=None, op0=ALU.mult,
            )
            # bw init: copy E bw slice of rotation 0
            nc.vector.tensor_copy(st[:, NW:W2], ebuf[:, NW:W2])

            def renorm(state, slot_fw, slot_bw):
                cs = m_ps.tile([1, W2], fp32, tag="m")
                nc.tensor.matmul(cs, ones_col_bf, state, start=True, stop=True)
                # park: bw all cols, fw only k=0 cols
                nc.vector.tensor_copy(
                    glog[:, slot_bw * NW:(slot_bw + 1) * NW], cs[:, NW:W2]
                )
                nc.vector.tensor_copy(
                    glog[:, 3 * NW + slot_fw * BC:3 * NW + (slot_fw + 1) * BC],
                    cs[:, 0:BC],
                )
                rec = smallp.tile([1, W2], fp32, tag="rec")
                nc.vector.reciprocal(rec, cs)
                bc_ps = m_ps.tile([128, W2], fp32, tag="m")
                nc.tensor.matmul(bc_ps, ones_row_f, rec, start=True, stop=True)
                out = statep.tile([128, W2], bf16, tag="st")
                nc.vector.tensor_tensor(out=out, in0=state, in1=bc_ps, op=ALU.mult)
                return out

            # ---------------- rotation loop ----------------
            nren = 0
            for r in range(1, L):
                q = q_ps.tile([128, 512], fp32, tag="q")
                nc.tensor.matmul(q[:, 0:NW], w_bf, st[:, 0:NW], start=True, stop=True)
                nc.tensor.matmul(q[:, NW:W2], wt_bf, st[:, NW:W2], start=True, stop=True)
                nst = statep.tile([128, W2], bf16, tag="st")
                nc.vector.tensor_tensor(
                    out=nst, in0=q[:, 0:W2], in1=ebuf[:, r * W2:(r + 1) * W2],
                    op=ALU.mult,
                )
                st = nst
                if r in RENORMS:
                    st = renorm(st, nren, nren)
                    nren += 1
                # E prefetch (one chunk ahead of consumption)
                if r % ECH == 0 and r // ECH + 1 < L // ECH:
                    emit_echunk(r // ECH + 1)
                # gold: 2 units per rotation, DMA 2 rotations ahead,
                # one-hot build 1 rotation ahead
                u_dma = 2 * (r + 1)
                if u_dma % 4 == 0 and u_dma // 4 < NUNITS // 4:
                    gold_dma(u_dma // 4)
                for u in (2 * r, 2 * r + 1):
                    if u < NUNITS:
                        gold_build(u)
                for u in (2 * (r - 1), 2 * (r - 1) + 1):
                    if u >= 0 and u < NUNITS:
                        gold_mm(u)

            # drain remaining gold units
            u0 = 2 * (L - 1)
            for ri in range(2 * L // 4 + 1, NUNITS // 4):
                gold_dma(ri)
            for u in range(u0, NUNITS):
                if u not in ohs:
                    gold_build(u)
            for u in range(u0, NUNITS):
                gold_mm(u)

            # ---------------- epilogue ----------------
            # bw final matmul: g_k = W h_k
            gfin = q_ps.tile([128, 512], fp32, tag="q")
            nc.tensor.matmul(gfin[:, 0:NW], wt_bf, st[:, NW:W2], start=True, stop=True)
            # couplings: cpl[:, c] = f[:, c] * g[:, c]
            cpl = dumpp.tile([128, NW], bf16, tag="cpl")
            nc.vector.tensor_tensor(
                out=cpl, in0=gfin[:, 0:NW], in1=st[:, 0:NW], op=ALU.mult
            )
            csum = m_ps.tile([1, W2], fp32, tag="m")
            nc.tensor.matmul(csum[:, 0:NW], ones_col_bf, cpl, start=True, stop=True)
            nc.tensor.matmul(csum[:, NW:W2], ones_col_bf, st[:, 0:NW], start=True, stop=True)
            # ln sums (ACT with accumulate)
            lncpl = smallp.tile([1, NW], fp32, tag="lncpl")
            acc_cpl = consts.tile([1, 1], fp32)
            nc.scalar.activation(lncpl, csum[:, 0:NW], AF.Ln, accum_out=acc_cpl)
            lnfs = smallp.tile([1, NW - BC], fp32, tag="lnfs")
            acc_fs = consts.tile([1, 1], fp32)
            nc.scalar.activation(
                lnfs, csum[:, NW + BC:W2], AF.Ln, accum_out=acc_fs
            )
            lnglog = smallp.tile([1, 3 * NW + 3 * BC], fp32, tag="lnglog")
            acc_gl = consts.tile([1, 1], fp32)
            nc.scalar.activation(lnglog, glog, AF.Ln, accum_out=acc_gl)

            # gold finalize
            cdump = dumpp.tile([128, 256], fp32, tag="cdump")
            nc.scalar.activation(cdump, gold_ps[:, 0:256], AF.Copy)
            cmul = dumpp.tile([128, 256], fp32, tag="cmul")
            nc.vector.tensor_tensor(out=cmul, in0=cdump, in1=tri, op=ALU.mult)
            rowred = smallp.tile([128, 1], fp32, tag="rowred")
            nc.vector.tensor_reduce(rowred, cmul, axis=AX.X, op=ALU.add)
            goldtot_ps = m_ps.tile([1, 1], fp32, tag="m")
            nc.tensor.matmul(goldtot_ps, ones_col_f, rowred, start=True, stop=True)

            # loss_sum = acc_cpl - acc_fs + acc_gl + chat16 - goldtot
            res = smallp.tile([1, 1], fp32, tag="res")
            nc.vector.tensor_tensor(out=res, in0=acc_cpl, in1=acc_fs, op=ALU.subtract)
            nc.vector.tensor_tensor(out=res, in0=res, in1=acc_gl, op=ALU.add)
            nc.vector.tensor_tensor(out=res, in0=res, in1=chat16, op=ALU.add)
            nc.vector.tensor_tensor(out=res, in0=res, in1=goldtot_ps, op=ALU.subtract)
            nc.sync.dma_start(out=out_d[:], in_=res[0:1, :])

    return nc


def _get_compiled(finalized=False):
    global _compiled
    if _compiled is None:
        try:
            _compiled = _build_program(use_gpsimd=True)
        except Exception:
            _compiled = _build_program(use_gpsimd=False)
    if finalized and not _compiled.is_finalized():
        _compiled.finalize()
    return _compiled


def make_in_maps(emissions, transitions, tags):
    bf = ml_dtypes.bfloat16
    in_maps = []
    tr32 = np.ascontiguousarray(transitions, dtype=np.float32)
    for c in range(NCORES):
        sl = slice(c * BC, (c + 1) * BC)
        em = np.asarray(emissions[sl], dtype=np.float32)
        em_bf = em.astype(bf)                       # [16, 2048, 128]
        em4 = em_bf.reshape(BC, K, L, T)            # [b, k, r, tag]
        # fw: [tag, r, k(0..14), b];  bw: [tag, r, k-1(1..15), b] reversed r
        efw = em4[:, 0:K - 1].transpose(3, 2, 1, 0)           # [tag, r, k, b]
        ebw = em4[:, 1:K, ::-1].transpose(3, 2, 1, 0)         # [tag, r, k-1, b]
        elay = np.concatenate(
            [efw.reshape(T, L, NW), ebw.reshape(T, L, NW)], axis=2
        ).reshape(T, L * 2 * NW)
        tg = np.asarray(tags[sl]).astype(np.float32)          # [16, 2048]
        tgsh = np.concatenate(
            [tg[:, 1:], np.full((BC, 1), JUNK_TAG, np.float32)], axis=1
        )
        tcols = tg.reshape(BC, NSB, 128).transpose(2, 1, 0).reshape(128, NSB * BC)
        tshcols = tgsh.reshape(BC, NSB, 128).transpose(2, 1, 0).reshape(128, NSB * BC)
        in_maps.append({
            "e_lay": np.ascontiguousarray(elay),
            "emis_gold": np.ascontiguousarray(em_bf),
            "transitions": tr32,
            "tag_cols": np.ascontiguousarray(tcols),
            "tagsh_cols": np.ascontiguousarray(tshcols),
        })
    return in_maps


def _run_device(emissions, transitions, tags):
    from concourse.bass_utils import run_bass_kernel_spmd

    nc = _get_compiled(finalized=True)
    res = run_bass_kernel_spmd(
        nc, make_in_maps(emissions, transitions, tags), list(range(NCORES))
    )
    tot = sum(float(res.results[c]["loss_parts"][0]) for c in range(NCORES))
    return np.float32(tot / B)


def _run_host(emissions, transitions, tags, mask):
    """Slow but fully general fallback (any mask pattern)."""
    e = emissions.astype(np.float64)
    t = transitions.astype(np.float64)

    def lse(x, axis):
        m = x.max(axis=axis, keepdims=True)
        return (m + np.log(np.exp(x - m).sum(axis=axis, keepdims=True))).squeeze(axis)

    score = e[:, 0]
    for s in range(1, e.shape[1]):
        nxt = lse(score[:, :, None] + t[None, :, :] + e[:, s, None, :], axis=1)
        score = np.where(mask[:, s, None], nxt, score)
    log_Z = lse(score, axis=1)
    emit = np.take_along_axis(e, tags[..., None].astype(np.int64), axis=2)[..., 0]
    trans_sc = t[tags[:, :-1].astype(np.int64), tags[:, 1:].astype(np.int64)]
    m = mask[:, 1:].astype(np.float64)
    seq = emit[:, 0] + ((trans_sc + emit[:, 1:]) * m).sum(axis=1)
    return np.float32((log_Z - seq).mean())


def kernel(emissions, transitions, tags, mask):
    emissions = np.asarray(emissions)
    transitions = np.asarray(transitions)
    tags = np.asarray(tags)
    mask = np.asarray(mask)
    if emissions.shape != (B, S, T) or not mask.all():
        return _run_host(emissions, transitions, tags, mask)
    return _run_device(emissions, transitions, tags)


# revision 9
# speedup vs baseline: 3.4665x; 3.4665x over previous
"""Trainium2 Bass kernel for a batched linear-chain CRF negative log-likelihood.

reference semantics (B=128, S=2048, T=128):
    forward algorithm over S steps -> log_Z per batch
    gold path score = emissions gathered at tags + transitions gathered at
    (tag_t, tag_{t+1}) pairs, summed over time
    output = mean(log_Z - seq_score)   (scalar f32)

Strategy (v3): segmented rank-1 forward algorithm.
  - data parallel over 8 cores: 16 batch rows per core, transitions replicated.
  - linear space: M_t = diag(E_t) W^T with W = exp(transitions),
    E_t = exp(emit_t - chat).  Z = 1^T M_{S-1}..M_1 a0,  a0 = E_0.
  - split S into K=16 segments of L=128.  Products of positive matrices
    contract to rank-1, so P_k ~ f_k g_k^T / s_k with f_k = P_k 1,
    g_k = P_k^T 1, s_k = 1^T P_k 1.  Then
      ln Z ~ sum_k ln(g_k . f_{k-1}) - sum_k ln(s_k) + parked renorm logs
    with segment 0 run on the true a0 and segment K-1 only backward.
    (validated vs fp64 reference: |err| < 0.06 absolute on lnZ ~ 12000.)
  - all 15 fw chains and 15 bw chains advance together: ONE [128x480]
    moving-operand matmul pair per rotation (fw cols 0:240 with stationary W,
    bw cols 240:480 with stationary W^T) into one PSUM bank, then ONE DVE
    multiply with a host-prelaid E slice (rotation-major layout) updates the
    joint state.  127 rotations instead of 2047 sequential steps.
  - gold path: per (b, sblock) one-hot count matmuls accumulate a single
    C|D PSUM region for the whole core (mean-only output); 2 units per
    rotation interleave into PE/DVE/GPSIMD idle time.
  - host prep: bf16 cast + layout permutations of emissions (E layouts in
    rotation-major order, gold in natural order), tag columns as f32.
"""

import numpy as np
import ml_dtypes

B, S, T = 128, 2048, 128
NCORES = 8
BC = B // NCORES        # 16 batch rows per core
L = 128                 # segment length (rotations)
K = S // L              # 16 segments
NCH = K - 1             # 15 chains per direction
NW = NCH * BC           # 240 state columns per direction
NSB = S // 128          # 16 s-blocks for gold
NUNITS = BC * NSB       # 256 gold units
JUNK_TAG = 60000.0
RENORMS = (42, 84, 126)
ECH = 4                 # E chunk: rotations per DMA (4 * 960 cols)

_compiled = None


def _build_program(use_gpsimd=True):
    import concourse.bass as bass
    import concourse.bacc as bacc
    import concourse.tile as tile
    from concourse import mybir
    from concourse.masks import make_identity

    fp32 = mybir.dt.float32
    bf16 = mybir.dt.bfloat16
    AF = mybir.ActivationFunctionType
    ALU = mybir.AluOpType
    AX = mybir.AxisListType

    nc = bacc.Bacc(None)
    # E layout: col r*480 + 0:240 = fw slice (k*16+b -> E[b, k*L+r, :]),
    #           col r*480 + 240:480 = bw slice ((k-1)*16+b -> E[b, (k+1)*L-1-r, :])
    e_d = nc.declare_dram_parameter("e_lay", [128, L * 2 * NW], bf16, isOutput=False)
    eg_d = nc.declare_dram_parameter("emis_gold", [BC, S, T], bf16, isOutput=False)
    tr_d = nc.declare_dram_parameter("transitions", [T, T], fp32, isOutput=False)
    tc_d = nc.declare_dram_parameter("tag_cols", [128, NSB * BC], fp32, isOutput=False)
    ts_d = nc.declare_dram_parameter("tagsh_cols", [128, NSB * BC], fp32, isOutput=False)
    out_d = nc.declare_dram_parameter("loss_parts", [1], fp32, isOutput=True)

    W2 = 2 * NW  # 480

    with tile.TileContext(nc) as tc:
        with (
            tc.tile_pool(name="consts", bufs=1) as consts,
            tc.tile_pool(name="ebuf", bufs=1) as ebufp,
            tc.tile_pool(name="estg", bufs=3) as estgp,
            tc.tile_pool(name="state", bufs=3) as statep,
            tc.tile_pool(name="ring", bufs=3) as ringp,
            tc.tile_pool(name="oh", bufs=3) as ohp,
            tc.tile_pool(name="small", bufs=4) as smallp,
            tc.tile_pool(name="dump", bufs=2) as dumpp,
            tc.tile_pool(name="q_ps", bufs=2, space="PSUM") as q_ps,
            tc.tile_pool(name="g_ps", bufs=1, space="PSUM") as g_ps,
            tc.tile_pool(name="m_ps", bufs=2, space="PSUM") as m_ps,
            tc.tile_pool(name="tp_ps", bufs=1, space="PSUM") as tp_ps,
        ):
            # ---------------- constants ----------------
            ident = consts.tile([128, 128], fp32)
            make_identity(nc, ident)
            ident_bf = consts.tile([128, 128], bf16)
            make_identity(nc, ident_bf)
            iota = consts.tile([128, 128], bf16)
            nc.gpsimd.iota(
                iota, pattern=[[1, 128]], base=0, channel_multiplier=0,
                allow_small_or_imprecise_dtypes=True,
            )
            ones_col_bf = consts.tile([128, 1], bf16)
            nc.vector.memset(ones_col_bf, 1.0)
            ones_col_f = consts.tile([128, 1], fp32)
            nc.vector.memset(ones_col_f, 1.0)
            ones_row_f = consts.tile([1, 128], fp32)
            nc.vector.memset(ones_row_f, 1.0)

            tr_sb = consts.tile([128, 128], fp32)
            nc.sync.dma_start(out=tr_sb, in_=tr_d[:, :])
            tag_cols = consts.tile([128, NSB * BC], fp32)
            nc.sync.dma_start(out=tag_cols, in_=tc_d[:, :])
            tagsh_cols = consts.tile([128, NSB * BC], fp32)
            nc.sync.dma_start(out=tagsh_cols, in_=ts_d[:, :])

            w_bf = consts.tile([128, 128], bf16)
            nc.scalar.activation(w_bf, tr_sb, AF.Exp)
            wt_psum = tp_ps.tile([128, 128], bf16, tag="tp")
            nc.tensor.transpose(wt_psum, w_bf, ident_bf)
            wt_bf = consts.tile([128, 128], bf16)
            nc.vector.tensor_copy(wt_bf, wt_psum)

            # [trans | identity] for the gold finalize
            tri = consts.tile([128, 256], fp32)
            nc.vector.tensor_copy(tri[:, 0:128], tr_sb)
            nc.vector.tensor_copy(tri[:, 128:256], ident)

            # rho = W^T 1 (colsums of W) as [128,1];  chat = mean_{j>=1} ln rho_j
            rho_ps = m_ps.tile([128, 1], fp32, tag="m")
            nc.tensor.matmul(rho_ps, w_bf, ones_col_bf, start=True, stop=True)
            rho = consts.tile([128, 1], fp32)
            nc.vector.tensor_copy(rho, rho_ps)
            colw_ps = m_ps.tile([1, 128], fp32, tag="m")
            nc.tensor.matmul(colw_ps, ones_col_bf, w_bf, start=True, stop=True)
            lncol = smallp.tile([1, 127], fp32, tag="lncol")
            lnsum = consts.tile([1, 1], fp32)
            nc.scalar.activation(lncol, colw_ps[:, 1:128], AF.Ln, accum_out=lnsum)
            chat16 = consts.tile([1, 1], fp32)
            # 16 batch rows * S * chat  (chat = lnsum/127)
            nc.scalar.activation(chat16, lnsum, AF.Copy, scale=float(BC) * float(S) / 127.0)
            negchat = consts.tile([1, 1], fp32)
            nc.scalar.activation(negchat, lnsum, AF.Copy, scale=-1.0 / 127.0)
            nbc_ps = m_ps.tile([128, 1], fp32, tag="m")
            nc.tensor.matmul(nbc_ps, ones_row_f, negchat, start=True, stop=True)
            negchat_bc = consts.tile([128, 1], fp32)
            nc.vector.tensor_copy(negchat_bc, nbc_ps)

            # parked renorm logs: bw 3x240 | fw(k=0 only) 3x16
            glog = consts.tile([1, 3 * NW + 3 * BC], fp32)
            nc.vector.memset(glog, 1.0)

            # ---------------- E supply (DMA + exp) ----------------
            ebuf = ebufp.tile([128, L * W2], bf16)  # 128 x 61440, 120KB/part

            def emit_echunk(c):
                # rotations [c*ECH, (c+1)*ECH)
                cols = ECH * W2
                stg = estgp.tile([128, cols], bf16, tag="estg")
                nc.sync.dma_start(out=stg, in_=e_d[:, c * cols:(c + 1) * cols])
                nc.scalar.activation(
                    ebuf[:, c * cols:(c + 1) * cols], stg, AF.Exp, bias=negchat_bc
                )

            emit_echunk(0)
            emit_echunk(1)

            # ---------------- gold machinery ----------------
            gold_ps = g_ps.tile([128, 512], fp32, tag="gold")

            iota4 = consts.tile([128, 512], bf16)
            for _i in range(4):
                nc.vector.tensor_copy(iota4[:, _i * 128:(_i + 1) * 128], iota)

            # ring ri covers units 4ri..4ri+3 (same b, 4 consecutive sblocks):
            # ring cols [ohsh0..ohsh3 | emis0..emis3], oh-quad separate tile
            rings = {}

            def _tag_bcast(tcols, ri):
                b = (4 * ri) // NSB
                sb0 = (4 * ri) % NSB
                ap = tcols.rearrange("p (sb b) -> p sb b", b=BC)
                ap = ap[:, sb0:sb0 + 4, b:b + 1]          # [128, 4, 1]
                return ap.broadcast_to([128, 4, 128])

            def gold_dma(ri):
                b = (4 * ri) // NSB
                sb0 = (4 * ri) % NSB
                ring = ringp.tile([128, 1024], bf16, tag="ring", name=f"ring_{ri}")
                src = eg_d[b, sb0 * 128:(sb0 + 4) * 128, :].rearrange(
                    "(u p) t -> p u t", u=4
                )
                dst = ring[:, 512:1024].rearrange("p (u t) -> p u t", u=4)
                nc.sync.dma_start(out=dst, in_=src)
                rings[ri] = ring

            def gold_build(ri):
                # quad one-hots: oh (stationary) + ohsh (into ring cols 0:512)
                ring = rings[ri]
                ohq = ohp.tile([128, 512], bf16, tag="ohq", name=f"ohq_{ri}")
                nc.vector.tensor_tensor(
                    out=ohq.rearrange("p (u t) -> p u t", u=4),
                    in0=iota4.rearrange("p (u t) -> p u t", u=4),
                    in1=_tag_bcast(tag_cols, ri), op=ALU.is_equal,
                )
                nc.vector.tensor_tensor(
                    out=ring[:, 0:512].rearrange("p (u t) -> p u t", u=4),
                    in0=iota4.rearrange("p (u t) -> p u t", u=4),
                    in1=_tag_bcast(tagsh_cols, ri), op=ALU.is_equal,
                )
                rings[ri] = (ring, ohq)

            def gold_mm(u):
                ri, q = u // 4, u % 4
                ring, ohq = rings[ri]
                mv = ring.rearrange("p (h x) -> p h x", h=2)[:, :, q * 128:(q + 1) * 128]
                nc.tensor.matmul(
                    gold_ps[:, 0:256], ohq[:, q * 128:(q + 1) * 128], mv,
                    start=(u == 0), stop=(u == NUNITS - 1),
                )
                if q == 3:
                    rings.pop(ri)

            # prime gold pipeline: rings 0,1 DMA'd; ring 0 one-hot quads built
            gold_dma(0)
            gold_dma(1)
            gold_build(0)

            # ---------------- chain states ----------------
            # joint state [128 x 480]: fw cols 0:240 (chain k*16+b covers seg k,
            # k=0..14), bw cols 240:480 ((k-1)*16+b covers seg k, k=1..15)
            st = statep.tile([128, W2], bf16, tag="st")
            # fw init: seg0 = E[:,0:16]; interior = E * rho
            nc.vector.tensor_copy(st[:, 0:BC], ebuf[:, 0:BC])
            nc.vector.tensor_scalar(
                out=st[:, BC:NW], in0=ebuf[:, BC:NW], scalar1=rho,
                scalar2=None, op0=ALU.mult,
            )
            # bw init: copy E bw slice of rotation 0
            nc.vector.tensor_copy(st[:, NW:W2], ebuf[:, NW:W2])

            def renorm(state, slot_fw, slot_bw):
                cs = m_ps.tile([1, W2], fp32, tag="m")
                nc.tensor.matmul(cs, ones_col_bf, state, start=True, stop=True)
                # park: bw all cols, fw only k=0 cols
                nc.vector.tensor_copy(
                    glog[:, slot_bw * NW:(slot_bw + 1) * NW], cs[:, NW:W2]
                )
                nc.vector.tensor_copy(
                    glog[:, 3 * NW + slot_fw * BC:3 * NW + (slot_fw + 1) * BC],
                    cs[:, 0:BC],
                )
                rec = smallp.tile([1, W2], fp32, tag="rec")
                nc.vector.reciprocal(rec, cs)
                bc_ps = m_ps.tile([128, W2], fp32, tag="m")
                nc.tensor.matmul(bc_ps, ones_row_f, rec, start=True, stop=True)
                out = statep.tile([128, W2], bf16, tag="st")
                nc.vector.tensor_tensor(out=out, in0=state, in1=bc_ps, op=ALU.mult)
                return out

            # ---------------- rotation loop ----------------
            nren = 0
            for r in range(1, L):
                q = q_ps.tile([128, 512], fp32, tag="q")
                nc.tensor.matmul(q[:, 0:NW], w_bf, st[:, 0:NW], start=True, stop=True)
                nc.tensor.matmul(q[:, NW:W2], wt_bf, st[:, NW:W2], start=True, stop=True)
                nst = statep.tile([128, W2], bf16, tag="st")
                nc.vector.tensor_tensor(
                    out=nst, in0=q[:, 0:W2], in1=ebuf[:, r * W2:(r + 1) * W2],
                    op=ALU.mult,
                )
                st = nst
                if r in RENORMS:
                    st = renorm(st, nren, nren)
                    nren += 1
                # E prefetch (one chunk ahead of consumption)
                if r % ECH == 0 and r // ECH + 1 < L // ECH:
                    emit_echunk(r // ECH + 1)
                # gold: 2 unit-mms per rotation; quad-build + DMA on odd r
                if r % 2 == 1:
                    if (r + 1) // 2 < NUNITS // 4:
                        gold_build((r + 1) // 2)
                    if (r + 3) // 2 < NUNITS // 4:
                        gold_dma((r + 3) // 2)
                for u in (2 * (r - 1), 2 * (r - 1) + 1):
                    if u < NUNITS:
                        gold_mm(u)

            # drain remaining gold units (254, 255)
            for u in range(2 * (L - 1), NUNITS):
                gold_mm(u)

            # ---------------- epilogue ----------------
            # bw final matmul: g_k = W h_k
            gfin = q_ps.tile([128, 512], fp32, tag="q")
            nc.tensor.matmul(gfin[:, 0:NW], wt_bf, st[:, NW:W2], start=True, stop=True)
            # couplings: cpl[:, c] = f[:, c] * g[:, c]
            cpl = dumpp.tile([128, NW], bf16, tag="cpl")
            nc.vector.tensor_tensor(
                out=cpl, in0=gfin[:, 0:NW], in1=st[:, 0:NW], op=ALU.mult
            )
            csum = m_ps.tile([1, W2], fp32, tag="m")
            nc.tensor.matmul(csum[:, 0:NW], ones_col_bf, cpl, start=True, stop=True)
            nc.tensor.matmul(csum[:, NW:W2], ones_col_bf, st[:, 0:NW], start=True, stop=True)
            # ln sums (ACT with accumulate)
            lncpl = smallp.tile([1, NW], fp32, tag="lncpl")
            acc_cpl = consts.tile([1, 1], fp32)
            nc.scalar.activation(lncpl, csum[:, 0:NW], AF.Ln, accum_out=acc_cpl)
            lnfs = smallp.tile([1, NW - BC], fp32, tag="lnfs")
            acc_fs = consts.tile([1, 1], fp32)
            nc.scalar.activation(
                lnfs, csum[:, NW + BC:W2], AF.Ln, accum_out=acc_fs
            )
            lnglog = smallp.tile([1, 3 * NW + 3 * BC], fp32, tag="lnglog")
            acc_gl = consts.tile([1, 1], fp32)
            nc.scalar.activation(lnglog, glog, AF.Ln, accum_out=acc_gl)

            # gold finalize
            cdump = dumpp.tile([128, 256], fp32, tag="cdump")
            nc.scalar.activation(cdump, gold_ps[:, 0:256], AF.Copy)
            cmul = dumpp.tile([128, 256], fp32, tag="cmul")
            nc.vector.tensor_tensor(out=cmul, in0=cdump, in1=tri, op=ALU.mult)
            rowred = smallp.tile([128, 1], fp32, tag="rowred")
            nc.vector.tensor_reduce(rowred, cmul, axis=AX.X, op=ALU.add)
            goldtot_ps = m_ps.tile([1, 1], fp32, tag="m")
            nc.tensor.matmul(goldtot_ps, ones_col_f, rowred, start=True, stop=True)

            # loss_sum = acc_cpl - acc_fs + acc_gl + chat16 - goldtot
            res = smallp.tile([1, 1], fp32, tag="res")
            nc.vector.tensor_tensor(out=res, in0=acc_cpl, in1=acc_fs, op=ALU.subtract)
            nc.vector.tensor_tensor(out=res, in0=res, in1=acc_gl, op=ALU.add)
            nc.vector.tensor_tensor(out=res, in0=res, in1=chat16, op=ALU.add)
            nc.vector.tensor_tensor(out=res, in0=res, in1=goldtot_ps, op=ALU.subtract)
            nc.sync.dma_start(out=out_d[:], in_=res[0:1, :])

    return nc


def _get_compiled(finalized=False):
    global _compiled
    if _compiled is None:
        try:
            _compiled = _build_program(use_gpsimd=True)
        except Exception:
            _compiled = _build_program(use_gpsimd=False)
    if finalized and not _compiled.is_finalized():
        _compiled.finalize()
    return _compiled


def make_in_maps(emissions, transitions, tags):
    bf = ml_dtypes.bfloat16
    in_maps = []
    tr32 = np.ascontiguousarray(transitions, dtype=np.float32)
    for c in range(NCORES):
        sl = slice(c * BC, (c + 1) * BC)
        em = np.asarray(emissions[sl], dtype=np.float32)
        em_bf = em.astype(bf)                       # [16, 2048, 128]
        em4 = em_bf.reshape(BC, K, L, T)            # [b, k, r, tag]
        # fw: [tag, r, k(0..14), b];  bw: [tag, r, k-1(1..15), b] reversed r
        efw = em4[:, 0:K - 1].transpose(3, 2, 1, 0)           # [tag, r, k, b]
        ebw = em4[:, 1:K, ::-1].transpose(3, 2, 1, 0)         # [tag, r, k-1, b]
        elay = np.concatenate(
            [efw.reshape(T, L, NW), ebw.reshape(T, L, NW)], axis=2
        ).reshape(T, L * 2 * NW)
        tg = np.asarray(tags[sl]).astype(np.float32)          # [16, 2048]
        tgsh = np.concatenate(
            [tg[:, 1:], np.full((BC, 1), JUNK_TAG, np.float32)], axis=1
        )
        tcols = tg.reshape(BC, NSB, 128).transpose(2, 1, 0).reshape(128, NSB * BC)
        tshcols = tgsh.reshape(BC, NSB, 128).transpose(2, 1, 0).reshape(128, NSB * BC)
        in_maps.append({
            "e_lay": np.ascontiguousarray(elay),
            "emis_gold": np.ascontiguousarray(em_bf),
            "transitions": tr32,
            "tag_cols": np.ascontiguousarray(tcols),
            "tagsh_cols": np.ascontiguousarray(tshcols),
        })
    return in_maps


def _run_device(emissions, transitions, tags):
    from concourse.bass_utils import run_bass_kernel_spmd

    nc = _get_compiled(finalized=True)
    res = run_bass_kernel_spmd(
        nc, make_in_maps(emissions, transitions, tags), list(range(NCORES))
    )
    tot = sum(float(res.results[c]["loss_parts"][0]) for c in range(NCORES))
    return np.float32(tot / B)


def _run_host(emissions, transitions, tags, mask):
    """Slow but fully general fallback (any mask pattern)."""
    e = emissions.astype(np.float64)
    t = transitions.astype(np.float64)

    def lse(x, axis):
        m = x.max(axis=axis, keepdims=True)
        return (m + np.log(np.exp(x - m).sum(axis=axis, keepdims=True))).squeeze(axis)

    score = e[:, 0]
    for s in range(1, e.shape[1]):
        nxt = lse(score[:, :, None] + t[None, :, :] + e[:, s, None, :], axis=1)
        score = np.where(mask[:, s, None], nxt, score)
    log_Z = lse(score, axis=1)
    emit = np.take_along_axis(e, tags[..., None].astype(np.int64), axis=2)[..., 0]
    trans_sc = t[tags[:, :-1].astype(np.int64), tags[:, 1:].astype(np.int64)]
    m = mask[:, 1:].astype(np.float64)
    seq = emit[:, 0] + ((trans_sc + emit[:, 1:]) * m).sum(axis=1)
    return np.float32((log_Z - seq).mean())


def kernel(emissions, transitions, tags, mask):
    emissions = np.asarray(emissions)
    transitions = np.asarray(transitions)
    tags = np.asarray(tags)
    mask = np.asarray(mask)
    if emissions.shape != (B, S, T) or not mask.all():
        return _run_host(emissions, transitions, tags, mask)
    return _run_device(emissions, transitions, tags)


# revision 16
# speedup vs baseline: 3.5414x; 1.0216x over previous
"""Trainium2 Bass kernel for a batched linear-chain CRF negative log-likelihood.

reference semantics (B=128, S=2048, T=128):
    forward algorithm over S steps -> log_Z per batch
    gold path score = emissions gathered at tags + transitions gathered at
    (tag_t, tag_{t+1}) pairs, summed over time
    output = mean(log_Z - seq_score)   (scalar f32)

Strategy (v6): segmented rank-1 forward algorithm, K=32 segments.
  - data parallel over 8 cores: 16 batch rows per core, transitions replicated.
  - linear space: M_t = diag(E_t) W^T with W = exp(transitions),
    E_t = exp(emit_t - chat).  Z = 1^T M_{S-1}..M_1 a0,  a0 = E_0.
  - split S into K=32 segments of L=64.  Products of positive matrices
    contract to rank-1, so P_k ~ f_k g_k^T / s_k with f_k = P_k 1,
    g_k = P_k^T 1, s_k = 1^T P_k 1.  Then
      ln Z ~ sum_k ln(g_k . f_{k-1}) - sum_k ln s_k + parked renorm logs
    with segment 0 run on the true a0 and segment K-1 only backward.
    (validated vs fp64 reference: |err| < 0.06 absolute on lnZ ~ 12000.)
  - all 31 fw and 31 bw chains advance together: two 496-col matmuls
    (stationary W / W^T) into one padded 2-bank PSUM tile
    [fw 496|pad16|bw 496|pad16], then ONE DVE multiply with a
    host-prelaid E slice (rotation-major, same padded layout) updates the
    joint state.  63 rotations instead of 2047 sequential steps.
  - gold path: per (b, sblock) one-hot count matmuls accumulate a single
    C|D PSUM region for the whole core (mean-only output); one-hots are
    built 8-at-a-time ("octs") with a single broadcast-AP is_equal per
    oct; 4 unit-matmuls interleave per rotation into PE idle time.
  - host prep: bf16 cast + layout permutations of emissions (E layouts in
    rotation-major padded order, gold in natural order), tag columns f32.
"""

import numpy as np
import ml_dtypes

B, S, T = 128, 2048, 128
NCORES = 8
BC = B // NCORES        # 16 batch rows per core
L = 64                  # segment length (rotations)
K = S // L              # 32 segments
NCH = K - 1             # 31 chains per direction
NW = NCH * BC           # 496 state columns per direction
PAD = 16
BLK = 2 * (NW + PAD)    # 1024: padded per-rotation block
NSB = S // 128          # 16 s-blocks for gold
NUNITS = BC * NSB       # 256 gold units
RU = 8                  # gold units per ring/oct
JUNK_TAG = 60000.0
RENORMS = (21, 42, 63)
ECH = 2                 # E chunk: rotations per DMA

_compiled = None


def _build_program(use_gpsimd=True):
    import concourse.bass as bass
    import concourse.bacc as bacc
    import concourse.tile as tile
    from concourse import mybir
    from concourse.masks import make_identity

    fp32 = mybir.dt.float32
    bf16 = mybir.dt.bfloat16
    AF = mybir.ActivationFunctionType
    ALU = mybir.AluOpType
    AX = mybir.AxisListType

    nc = bacc.Bacc(None)
    e_d = nc.declare_dram_parameter("e_lay", [128, L * BLK], bf16, isOutput=False)
    eg_d = nc.declare_dram_parameter("emis_gold", [BC, S, T], bf16, isOutput=False)
    tr_d = nc.declare_dram_parameter("transitions", [T, T], fp32, isOutput=False)
    tc_d = nc.declare_dram_parameter("tag_cols", [128, NSB * BC], fp32, isOutput=False)
    ts_d = nc.declare_dram_parameter("tagsh_cols", [128, NSB * BC], fp32, isOutput=False)
    out_d = nc.declare_dram_parameter("loss_parts", [1], fp32, isOutput=True)

    B0, B1 = 0, NW + PAD          # fw / bw col offsets in the padded block

    with tile.TileContext(nc) as tc:
        with (
            tc.tile_pool(name="consts", bufs=1) as consts,
            tc.tile_pool(name="ebuf", bufs=1) as ebufp,
            tc.tile_pool(name="estg", bufs=2) as estgp,
            tc.tile_pool(name="state", bufs=3) as statep,
            tc.tile_pool(name="ring", bufs=3) as ringp,
            tc.tile_pool(name="oh", bufs=3) as ohp,
            tc.tile_pool(name="small", bufs=1) as smallp,
            tc.tile_pool(name="dump", bufs=1) as dumpp,
            tc.tile_pool(name="q_ps", bufs=2, space="PSUM") as q_ps,
            tc.tile_pool(name="g_ps", bufs=1, space="PSUM") as g_ps,
            tc.tile_pool(name="m_ps", bufs=1, space="PSUM") as m_ps,
        ):
            # ---------------- constants ----------------
            ident = consts.tile([128, 128], fp32)
            make_identity(nc, ident)
            ident_bf = consts.tile([128, 128], bf16)
            make_identity(nc, ident_bf)
            iota = consts.tile([128, 128], bf16)
            nc.gpsimd.iota(
                iota, pattern=[[1, 128]], base=0, channel_multiplier=0,
                allow_small_or_imprecise_dtypes=True,
            )
            ones_col_bf = consts.tile([128, 1], bf16)
            nc.vector.memset(ones_col_bf, 1.0)
            ones_col_f = consts.tile([128, 1], fp32)
            nc.vector.memset(ones_col_f, 1.0)
            ones_row_f = consts.tile([1, 128], fp32)
            nc.vector.memset(ones_row_f, 1.0)

            tr_sb = consts.tile([128, 128], fp32)
            nc.sync.dma_start(out=tr_sb, in_=tr_d[:, :])
            tag_cols = consts.tile([128, NSB * BC], fp32)
            nc.sync.dma_start(out=tag_cols, in_=tc_d[:, :])
            tagsh_cols = consts.tile([128, NSB * BC], fp32)
            nc.sync.dma_start(out=tagsh_cols, in_=ts_d[:, :])

            w_bf = consts.tile([128, 128], bf16)
            nc.scalar.activation(w_bf, tr_sb, AF.Exp)
            wt_psum = m_ps.tile([128, 128], bf16, tag="m")
            nc.tensor.transpose(wt_psum, w_bf, ident_bf)
            wt_bf = consts.tile([128, 128], bf16)
            nc.vector.tensor_copy(wt_bf, wt_psum)

            # [trans | identity] for the gold finalize
            tri = consts.tile([128, 256], fp32)
            nc.vector.tensor_copy(tri[:, 0:128], tr_sb)
            nc.vector.tensor_copy(tri[:, 128:256], ident)

            # rho = W^T 1 (colsums of W) as [128,1];  chat = mean_{j>=1} ln rho_j
            rho_ps = m_ps.tile([128, 1], fp32, tag="m")
            nc.tensor.matmul(rho_ps, w_bf, ones_col_bf, start=True, stop=True)
            rho = consts.tile([128, 1], fp32)
            nc.vector.tensor_copy(rho, rho_ps)
            colw_ps = m_ps.tile([1, 128], fp32, tag="m")
            nc.tensor.matmul(colw_ps, ones_col_bf, w_bf, start=True, stop=True)
            lncol = smallp.tile([1, 127], fp32, tag="lncol")
            lnsum = consts.tile([1, 1], fp32)
            nc.scalar.activation(lncol, colw_ps[:, 1:128], AF.Ln, accum_out=lnsum)
            chat16 = consts.tile([1, 1], fp32)
            nc.scalar.activation(chat16, lnsum, AF.Copy, scale=float(BC) * float(S) / 127.0)
            negchat = consts.tile([1, 1], fp32)
            nc.scalar.activation(negchat, lnsum, AF.Copy, scale=-1.0 / 127.0)
            nbc_ps = m_ps.tile([128, 1], fp32, tag="m")
            nc.tensor.matmul(nbc_ps, ones_row_f, negchat, start=True, stop=True)
            negchat_bc = consts.tile([128, 1], fp32)
            nc.vector.tensor_copy(negchat_bc, nbc_ps)

            # parked renorm logs: bw 3xNW | fw(k=0 only) 3xBC
            glog = consts.tile([1, 3 * NW + 3 * BC], fp32)
            nc.vector.memset(glog, 1.0)

            # ---------------- E supply (DMA + exp) ----------------
            ebuf = ebufp.tile([128, L * BLK], bf16)  # 128KB/partition

            def emit_echunk(c):
                cols = ECH * BLK
                stg = estgp.tile([128, cols], bf16, tag="estg")
                nc.sync.dma_start(out=stg, in_=e_d[:, c * cols:(c + 1) * cols])
                nc.scalar.activation(
                    ebuf[:, c * cols:(c + 1) * cols], stg, AF.Exp, bias=negchat_bc
                )

            emit_echunk(0)
            emit_echunk(1)

            # ---------------- gold machinery ----------------
            gold_ps = g_ps.tile([128, 512], fp32, tag="gold")

            iota8 = consts.tile([128, RU * 128], bf16)
            for _i in range(RU):
                nc.vector.tensor_copy(iota8[:, _i * 128:(_i + 1) * 128], iota)

            # ring ri covers units RU*ri..RU*ri+RU-1 (same b, RU consecutive
            # sblocks): ring cols [ohsh oct | emis oct], oh-oct separate tile
            rings = {}

            def _tag_bcast(tcols, ri):
                b = (RU * ri) // NSB
                sb0 = (RU * ri) % NSB
                ap = tcols.rearrange("p (sb b) -> p sb b", b=BC)
                ap = ap[:, sb0:sb0 + RU, b:b + 1]          # [128, RU, 1]
                return ap.broadcast_to([128, RU, 128])

            def gold_dma(ri):
                b = (RU * ri) // NSB
                sb0 = (RU * ri) % NSB
                ring = ringp.tile([128, 2 * RU * 128], bf16, tag="ring",
                                  name=f"ring_{ri}")
                src = eg_d[b, sb0 * 128:(sb0 + RU) * 128, :].rearrange(
                    "(u p) t -> p u t", u=RU
                )
                dst = ring[:, RU * 128:].rearrange("p (u t) -> p u t", u=RU)
                nc.sync.dma_start(out=dst, in_=src)
                rings[ri] = ring

            def gold_build(ri):
                # oct one-hots: oh (stationary) + ohsh (into ring left half)
                ring = rings[ri]
                ohq = ohp.tile([128, RU * 128], bf16, tag="ohq", name=f"ohq_{ri}")
                nc.vector.tensor_tensor(
                    out=ohq.rearrange("p (u t) -> p u t", u=RU),
                    in0=iota8.rearrange("p (u t) -> p u t", u=RU),
                    in1=_tag_bcast(tag_cols, ri), op=ALU.is_equal,
                )
                nc.vector.tensor_tensor(
                    out=ring[:, 0:RU * 128].rearrange("p (u t) -> p u t", u=RU),
                    in0=iota8.rearrange("p (u t) -> p u t", u=RU),
                    in1=_tag_bcast(tagsh_cols, ri), op=ALU.is_equal,
                )
                rings[ri] = (ring, ohq)

            def gold_mm(u):
                ri, q = u // RU, u % RU
                ring, ohq = rings[ri]
                mv = ring.rearrange("p (h x) -> p h x", h=2)[:, :, q * 128:(q + 1) * 128]
                nc.tensor.matmul(
                    gold_ps[:, 0:256], ohq[:, q * 128:(q + 1) * 128], mv,
                    start=(u == 0), stop=(u == NUNITS - 1),
                )
                if q == RU - 1:
                    rings.pop(ri)

            # prime gold pipeline
            gold_dma(0)
            gold_dma(1)
            gold_build(0)

            # ---------------- chain state ----------------
            # joint padded state [128 x 1024]:
            #   cols 0:496 fw (chain k*16+b covers seg k, k=0..30)
            #   cols 512:1008 bw ((k-1)*16+b covers seg k, k=1..31)
            st = statep.tile([128, BLK], bf16, tag="st")
            nc.vector.tensor_copy(st[:, B0:B0 + BC], ebuf[:, B0:B0 + BC])
            nc.vector.tensor_scalar(
                out=st[:, B0 + BC:B0 + NW], in0=ebuf[:, B0 + BC:B0 + NW],
                scalar1=rho, scalar2=None, op0=ALU.mult,
            )
            nc.vector.tensor_copy(st[:, B1:B1 + NW], ebuf[:, B1:B1 + NW])

            def renorm(state, slot):
                cs = m_ps.tile([1, BLK], fp32, tag="m")
                nc.tensor.matmul(cs[:, B0:B0 + NW], ones_col_bf,
                                 state[:, B0:B0 + NW], start=True, stop=True)
                nc.tensor.matmul(cs[:, B1:B1 + NW], ones_col_bf,
                                 state[:, B1:B1 + NW], start=True, stop=True)
                # park: bw all cols at slot; fw only k=0 cols
                nc.vector.tensor_copy(glog[:, slot * NW:(slot + 1) * NW],
                                      cs[:, B1:B1 + NW])
                nc.vector.tensor_copy(
                    glog[:, 3 * NW + slot * BC:3 * NW + (slot + 1) * BC],
                    cs[:, B0:B0 + BC],
                )
                rec = smallp.tile([1, BLK], fp32, tag="rec")
                nc.vector.reciprocal(rec[:, B0:B0 + NW], cs[:, B0:B0 + NW])
                nc.vector.reciprocal(rec[:, B1:B1 + NW], cs[:, B1:B1 + NW])
                bc_ps = m_ps.tile([128, BLK], fp32, tag="m")
                nc.tensor.matmul(bc_ps[:, B0:B0 + NW], ones_row_f,
                                 rec[:, B0:B0 + NW], start=True, stop=True)
                nc.tensor.matmul(bc_ps[:, B1:B1 + NW], ones_row_f,
                                 rec[:, B1:B1 + NW], start=True, stop=True)
                out = statep.tile([128, BLK], bf16, tag="st")
                nc.vector.tensor_tensor(
                    out=out[:, 0:B1 + NW], in0=state[:, 0:B1 + NW],
                    in1=bc_ps[:, 0:B1 + NW], op=ALU.mult,
                )
                return out

            # ---------------- rotation loop ----------------
            nren = 0
            for r in range(1, L):
                q = q_ps.tile([128, BLK], fp32, tag="q")
                nc.tensor.matmul(q[:, B0:B0 + NW], w_bf, st[:, B0:B0 + NW],
                                 start=True, stop=True)
                nc.tensor.matmul(q[:, B1:B1 + NW], wt_bf, st[:, B1:B1 + NW],
                                 start=True, stop=True)
                nst = statep.tile([128, BLK], bf16, tag="st")
                nc.vector.tensor_tensor(
                    out=nst[:, 0:B1 + NW], in0=q[:, 0:B1 + NW],
                    in1=ebuf[:, r * BLK:r * BLK + B1 + NW], op=ALU.mult,
                )
                st = nst
                if r in RENORMS:
                    st = renorm(st, nren)
                    nren += 1
                # E prefetch (one chunk ahead of consumption)
                if r % ECH == 0 and r // ECH + 1 < L // ECH:
                    emit_echunk(r // ECH + 1)
                # gold: 4 unit-mms per rotation; oct-build + DMA on odd r
                if r % 2 == 1:
                    if (r + 1) // 2 < NUNITS // RU:
                        gold_build((r + 1) // 2)
                    if (r + 3) // 2 < NUNITS // RU:
                        gold_dma((r + 3) // 2)
                for u in range(4 * (r - 1), 4 * r):
                    if u < NUNITS:
                        gold_mm(u)

            # drain remaining gold units
            for u in range(4 * (L - 1), NUNITS):
                gold_mm(u)

            # ---------------- epilogue ----------------
            # bw final matmul: g_k = W h_k
            gfin = q_ps.tile([128, BLK], fp32, tag="q")
            nc.tensor.matmul(gfin[:, B0:B0 + NW], wt_bf, st[:, B1:B1 + NW],
                             start=True, stop=True)
            # couplings: cpl[:, c] = f[:, c] * g[:, c]
            cpl = dumpp.tile([128, NW], bf16, tag="cpl")
            nc.vector.tensor_tensor(
                out=cpl, in0=gfin[:, B0:B0 + NW], in1=st[:, B0:B0 + NW],
                op=ALU.mult,
            )
            csum = m_ps.tile([1, BLK], fp32, tag="m")
            nc.tensor.matmul(csum[:, B0:B0 + NW], ones_col_bf, cpl,
                             start=True, stop=True)
            nc.tensor.matmul(csum[:, B1:B1 + NW], ones_col_bf,
                             st[:, B0:B0 + NW], start=True, stop=True)
            lncpl = smallp.tile([1, NW], fp32, tag="lncpl")
            acc_cpl = consts.tile([1, 1], fp32)
            nc.scalar.activation(lncpl, csum[:, B0:B0 + NW], AF.Ln,
                                 accum_out=acc_cpl)
            lnfs = smallp.tile([1, NW - BC], fp32, tag="lnfs")
            acc_fs = consts.tile([1, 1], fp32)
            nc.scalar.activation(lnfs, csum[:, B1 + BC:B1 + NW], AF.Ln,
                                 accum_out=acc_fs)
            lnglog = smallp.tile([1, 3 * NW + 3 * BC], fp32, tag="lnglog")
            acc_gl = consts.tile([1, 1], fp32)
            nc.scalar.activation(lnglog, glog, AF.Ln, accum_out=acc_gl)

            # gold finalize
            cdump = dumpp.tile([128, 256], fp32, tag="cdump")
            nc.scalar.activation(cdump, gold_ps[:, 0:256], AF.Copy)
            cmul = dumpp.tile([128, 256], fp32, tag="cmul")
            nc.vector.tensor_tensor(out=cmul, in0=cdump, in1=tri, op=ALU.mult)
            rowred = smallp.tile([128, 1], fp32, tag="rowred")
            nc.vector.tensor_reduce(rowred, cmul, axis=AX.X, op=ALU.add)
            goldtot_ps = m_ps.tile([1, 1], fp32, tag="m")
            nc.tensor.matmul(goldtot_ps, ones_col_f, rowred, start=True, stop=True)

            # loss_sum = acc_cpl - acc_fs + acc_gl + chat16 - goldtot
            res = smallp.tile([1, 1], fp32, tag="res")
            nc.vector.tensor_tensor(out=res, in0=acc_cpl, in1=acc_fs,
                                    op=ALU.subtract)
            nc.vector.tensor_tensor(out=res, in0=res, in1=acc_gl, op=ALU.add)
            nc.vector.tensor_tensor(out=res, in0=res, in1=chat16, op=ALU.add)
            nc.vector.tensor_tensor(out=res, in0=res, in1=goldtot_ps,
                                    op=ALU.subtract)
            nc.sync.dma_start(out=out_d[:], in_=res[0:1, :])

    return nc


def _get_compiled(finalized=False):
    global _compiled
    if _compiled is None:
        try:
            _compiled = _build_program(use_gpsimd=True)
        except Exception:
            _compiled = _build_program(use_gpsimd=False)
    if finalized and not _compiled.is_finalized():
        _compiled.finalize()
    return _compiled


def make_in_maps(emissions, transitions, tags):
    bf = ml_dtypes.bfloat16
    in_maps = []
    tr32 = np.ascontiguousarray(transitions, dtype=np.float32)
    for c in range(NCORES):
        sl = slice(c * BC, (c + 1) * BC)
        em = np.asarray(emissions[sl], dtype=np.float32)
        em_bf = em.astype(bf)                       # [16, 2048, 128]
        em4 = em_bf.reshape(BC, K, L, T)            # [b, k, r, tag]
        efw = em4[:, 0:K - 1].transpose(3, 2, 1, 0)           # [tag, r, k, b]
        ebw = em4[:, 1:K, ::-1].transpose(3, 2, 1, 0)         # [tag, r, k-1, b]
        elay = np.zeros((T, L, BLK), dtype=bf)
        elay[:, :, 0:NW] = efw.reshape(T, L, NW)
        elay[:, :, NW + PAD:NW + PAD + NW] = ebw.reshape(T, L, NW)
        tg = np.asarray(tags[sl]).astype(np.float32)          # [16, 2048]
        tgsh = np.concatenate(
            [tg[:, 1:], np.full((BC, 1), JUNK_TAG, np.float32)], axis=1
        )
        tcols = tg.reshape(BC, NSB, 128).transpose(2, 1, 0).reshape(128, NSB * BC)
        tshcols = tgsh.reshape(BC, NSB, 128).transpose(2, 1, 0).reshape(128, NSB * BC)
        in_maps.append({
            "e_lay": np.ascontiguousarray(elay.reshape(T, L * BLK)),
            "emis_gold": np.ascontiguousarray(em_bf),
            "transitions": tr32,
            "tag_cols": np.ascontiguousarray(tcols),
            "tagsh_cols": np.ascontiguousarray(tshcols),
        })
    return in_maps


def _run_device(emissions, transitions, tags):
    from concourse.bass_utils import run_bass_kernel_spmd

    nc = _get_compiled(finalized=True)
    res = run_bass_kernel_spmd(
        nc, make_in_maps(emissions, transitions, tags), list(range(NCORES))
    )
    tot = sum(float(res.results[c]["loss_parts"][0]) for c in range(NCORES))
    return np.float32(tot / B)


def _run_host(emissions, transitions, tags, mask):
    """Slow but fully general fallback (any mask pattern)."""
    e = emissions.astype(np.float64)
    t = transitions.astype(np.float64)

    def lse(x, axis):
        m = x.max(axis=axis, keepdims=True)
        return (m + np.log(np.exp(x - m).sum(axis=axis, keepdims=True))).squeeze(axis)

    score = e[:, 0]
    for s in range(1, e.shape[1]):
        nxt = lse(score[:, :, None] + t[None, :, :] + e[:, s, None, :], axis=1)
        score = np.where(mask[:, s, None], nxt, score)
    log_Z = lse(score, axis=1)
    emit = np.take_along_axis(e, tags[..., None].astype(np.int64), axis=2)[..., 0]
    trans_sc = t[tags[:, :-1].astype(np.int64), tags[:, 1:].astype(np.int64)]
    m = mask[:, 1:].astype(np.float64)
    seq = emit[:, 0] + ((trans_sc + emit[:, 1:]) * m).sum(axis=1)
    return np.float32((log_Z - seq).mean())


def kernel(emissions, transitions, tags, mask):
    emissions = np.asarray(emissions)
    transitions = np.asarray(transitions)
    tags = np.asarray(tags)
    mask = np.asarray(mask)
    if emissions.shape != (B, S, T) or not mask.all():
        return _run_host(emissions, transitions, tags, mask)
    return _run_device(emissions, transitions, tags)


# revision 19
# speedup vs baseline: 6.0058x; 1.6959x over previous
"""Trainium2 Bass kernel for a batched linear-chain CRF negative log-likelihood.

reference semantics (B=128, S=2048, T=128):
    forward algorithm over S steps -> log_Z per batch
    gold path score = emissions gathered at tags + transitions gathered at
    (tag_t, tag_{t+1}) pairs, summed over time
    output = mean(log_Z - seq_score)   (scalar f32)

Strategy (v7): segmented rank-1 forward algorithm, K=32 segments.
  - data parallel over 8 cores: 16 batch rows per core, transitions replicated.
  - linear space: M_t = diag(E_t) W^T with W = exp(transitions),
    E_t = exp(emit_t - chat).  Z = 1^T M_{S-1}..M_1 a0,  a0 = E_0.
  - split S into K=32 segments of L=64.  Products of positive matrices
    contract to rank-1, so P_k ~ f_k g_k^T / s_k with f_k = P_k 1,
    g_k = P_k^T 1, s_k = 1^T P_k 1.  Then
      ln Z ~ sum_k ln(g_k . f_{k-1}) - sum_k ln s_k + parked renorm logs
    with segment 0 run on the true a0 and segment K-1 only backward.
    (validated vs fp64 reference: |err| < 0.06 absolute on lnZ ~ 12000.)
  - all 31 fw and 31 bw chains advance together: one 496-col matmul per
    direction (stationary W / W^T) into its own PSUM bank, then one DVE
    multiply per direction with a host-prelaid E slice (rotation-major
    layout) updates that direction's state.  63 rotations instead of
    2047 sequential steps.  Renorm scales via ACT Reciprocal at r=31,63.
  - gold path: per (b, sblock) one-hot count matmuls accumulate a single
    C|D PSUM region for the whole core (mean-only output).  One-hots are
    HOST-ENCODED as fp8 (exact 0/1) and streamed from HBM in 8-unit ring
    blocks [oh oct | ohsh oct | emis oct]; 4 unit-matmuls interleave per
    rotation into PE idle time.  No on-device one-hot construction.
  - host prep: bf16 cast + chat subtraction + layout permutation for E,
    fp8 one-hot/emission encoding for gold.
"""

import numpy as np
import ml_dtypes

B, S, T = 128, 2048, 128
NCORES = 8
BC = B // NCORES        # 16 batch rows per core
L = 64                  # segment length (rotations)
K = S // L              # 32 segments
NCH = K - 1             # 31 chains per direction
NW = NCH * BC           # 496 state columns per direction
PAD = 16
BLK = 2 * (NW + PAD)    # 1024: padded per-rotation E block (fw | bw)
NSB = S // 128          # 16 s-blocks for gold
NUNITS = BC * NSB       # 256 gold units
RU = 8                  # gold units per ring
NRINGS = NUNITS // RU   # 32
RCOLS = 3 * RU * 128    # 3072: ring block cols [oh | ohsh | emis]
JUNK_TAG = 60000
RENORMS = (31, 63)
ECH = 2                 # E chunk: rotations per DMA

_compiled = None


def _build_program():
    import concourse.bass as bass
    import concourse.bacc as bacc
    import concourse.tile as tile
    from concourse import mybir
    from concourse.masks import make_identity

    fp32 = mybir.dt.float32
    bf16 = mybir.dt.bfloat16
    fp8 = mybir.dt.float8e4
    AF = mybir.ActivationFunctionType
    ALU = mybir.AluOpType
    AX = mybir.AxisListType

    nc = bacc.Bacc(None)
    e_d = nc.declare_dram_parameter("e_lay", [128, L * BLK], bf16, isOutput=False)
    g_d = nc.declare_dram_parameter("gold_lay", [128, NRINGS * RCOLS], fp8,
                                    isOutput=False)
    tr_d = nc.declare_dram_parameter("transitions", [T, T], fp32, isOutput=False)
    ch_d = nc.declare_dram_parameter("chat", [1], fp32, isOutput=False)
    out_d = nc.declare_dram_parameter("loss_parts", [1], fp32, isOutput=True)

    B1 = NW + PAD

    with tile.TileContext(nc) as tc:
        with (
            tc.tile_pool(name="consts", bufs=1) as consts,
            tc.tile_pool(name="ebuf", bufs=1) as ebufp,
            tc.tile_pool(name="estg", bufs=2) as estgp,
            tc.tile_pool(name="state", bufs=3) as statep,
            tc.tile_pool(name="ring", bufs=4) as ringp,
            tc.tile_pool(name="small", bufs=1) as smallp,
            tc.tile_pool(name="dump", bufs=1) as dumpp,
            tc.tile_pool(name="qf_ps", bufs=2, space="PSUM") as qf_ps,
            tc.tile_pool(name="qb_ps", bufs=2, space="PSUM") as qb_ps,
            tc.tile_pool(name="g_ps", bufs=1, space="PSUM") as g_ps,
            tc.tile_pool(name="m_ps", bufs=2, space="PSUM") as m_ps,
        ):
            # ---------------- constants ----------------
            ident = consts.tile([128, 128], fp32)
            make_identity(nc, ident)
            ident_bf = consts.tile([128, 128], bf16)
            make_identity(nc, ident_bf)
            ones_col_bf = consts.tile([128, 1], bf16)
            nc.vector.memset(ones_col_bf, 1.0)
            ones_col_f = consts.tile([128, 1], fp32)
            nc.vector.memset(ones_col_f, 1.0)
            ones_row_f = consts.tile([1, 128], fp32)
            nc.vector.memset(ones_row_f, 1.0)

            tr_sb = consts.tile([128, 128], fp32)
            nc.sync.dma_start(out=tr_sb, in_=tr_d[:, :])
            chat_sb = consts.tile([1, 1], fp32)
            nc.sync.dma_start(out=chat_sb, in_=ch_d[:])

            w_bf = consts.tile([128, 128], bf16)
            nc.scalar.activation(w_bf, tr_sb, AF.Exp)
            wt_psum = m_ps.tile([128, 128], bf16, tag="m")
            nc.tensor.transpose(wt_psum, w_bf, ident_bf)
            wt_bf = consts.tile([128, 128], bf16)
            nc.vector.tensor_copy(wt_bf, wt_psum)

            # [trans | identity] for the gold finalize
            tri = consts.tile([128, 256], fp32)
            nc.vector.tensor_copy(tri[:, 0:128], tr_sb)
            nc.vector.tensor_copy(tri[:, 128:256], ident)

            # rho = W^T 1 (colsums of W) as [128,1]
            rho_ps = m_ps.tile([128, 1], fp32, tag="m")
            nc.tensor.matmul(rho_ps, w_bf, ones_col_bf, start=True, stop=True)
            rho = consts.tile([128, 1], fp32)
            nc.vector.tensor_copy(rho, rho_ps)
            # BC * S * chat correction
            chat16 = consts.tile([1, 1], fp32)
            nc.scalar.activation(chat16, chat_sb, AF.Copy,
                                 scale=float(BC) * float(S))

            # parked renorm logs: bw 2xNW | fw(k=0 only) 2xBC
            NREN = len(RENORMS)
            glog = consts.tile([1, NREN * (NW + BC)], fp32)
            nc.vector.memset(glog, 1.0)

            # ---------------- E supply (DMA + exp) ----------------
            ebuf = ebufp.tile([128, L * BLK], bf16)  # 128KB/partition

            def emit_echunk(c):
                cols = ECH * BLK
                stg = estgp.tile([128, cols], bf16, tag="estg")
                nc.sync.dma_start(out=stg, in_=e_d[:, c * cols:(c + 1) * cols])
                nc.scalar.activation(ebuf[:, c * cols:(c + 1) * cols], stg, AF.Exp)

            emit_echunk(0)
            emit_echunk(1)

            # ---------------- gold machinery ----------------
            gold_ps = g_ps.tile([128, 512], fp32, tag="gold")
            rings = {}

            def gold_dma(ri):
                ring = ringp.tile([128, RCOLS], fp8, tag="ring", name=f"ring_{ri}")
                nc.sync.dma_start(out=ring, in_=g_d[:, ri * RCOLS:(ri + 1) * RCOLS])
                rings[ri] = ring

            def gold_mm(u):
                ri, q = u // RU, u % RU
                ring = rings[ri]
                mv = ring[:, RU * 128:].rearrange(
                    "p (h x) -> p h x", h=2)[:, :, q * 128:(q + 1) * 128]
                nc.tensor.matmul(
                    gold_ps[:, 0:256], ring[:, q * 128:(q + 1) * 128], mv,
                    start=(u == 0), stop=(u == NUNITS - 1),
                )
                if q == RU - 1:
                    rings.pop(ri)

            gold_dma(0)
            gold_dma(1)

            # ---------------- chain states ----------------
            # fst cols k*16+b cover seg k (k=0..30); bst (k-1)*16+b seg k (1..31)
            fst = statep.tile([128, NW], bf16, tag="fst")
            nc.vector.tensor_copy(fst[:, 0:BC], ebuf[:, 0:BC])
            nc.vector.tensor_scalar(
                out=fst[:, BC:NW], in0=ebuf[:, BC:NW], scalar1=rho,
                scalar2=None, op0=ALU.mult,
            )
            bst = statep.tile([128, NW], bf16, tag="bst")
            nc.vector.tensor_copy(bst, ebuf[:, B1:B1 + NW])

            def renorm(state, gcol0, gwidth, tag):
                cs = m_ps.tile([1, NW], fp32, tag="m")
                nc.tensor.matmul(cs, ones_col_bf, state, start=True, stop=True)
                nc.vector.tensor_copy(glog[:, gcol0:gcol0 + gwidth], cs[:, 0:gwidth])
                rec = smallp.tile([1, NW], fp32, tag="rec")
                nc.vector.reciprocal_approx_fast(out=rec, in_=cs)
                bc_ps = m_ps.tile([128, NW], fp32, tag="m")
                nc.tensor.matmul(bc_ps, ones_row_f, rec, start=True, stop=True)
                out = statep.tile([128, NW], bf16, tag=tag)
                nc.vector.tensor_tensor(out=out, in0=state, in1=bc_ps, op=ALU.mult)
                return out

            # ---------------- rotation loop ----------------
            nren = 0
            for r in range(1, L):
                qf = qf_ps.tile([128, 512], fp32, tag="qf")
                nc.tensor.matmul(qf[:, 0:NW], w_bf, fst, start=True, stop=True)
                qb = qb_ps.tile([128, 512], fp32, tag="qb")
                nc.tensor.matmul(qb[:, 0:NW], wt_bf, bst, start=True, stop=True)
                nfst = statep.tile([128, NW], bf16, tag="fst")
                nc.vector.tensor_tensor(
                    out=nfst, in0=qf[:, 0:NW],
                    in1=ebuf[:, r * BLK:r * BLK + NW], op=ALU.mult,
                )
                fst = nfst
                nbst = statep.tile([128, NW], bf16, tag="bst")
                nc.vector.tensor_tensor(
                    out=nbst, in0=qb[:, 0:NW],
                    in1=ebuf[:, r * BLK + B1:r * BLK + B1 + NW], op=ALU.mult,
                )
                bst = nbst
                if r in RENORMS:
                    fst = renorm(fst, NREN * NW + nren * BC, BC, "fst")
                    bst = renorm(bst, nren * NW, NW, "bst")
                    nren += 1
                # E prefetch (one chunk ahead of consumption)
                if r % ECH == 0 and r // ECH + 1 < L // ECH:
                    emit_echunk(r // ECH + 1)
                # gold: 4 unit-mms per rotation; ring DMA on odd r
                if r % 2 == 1 and (r + 3) // 2 < NRINGS:
                    gold_dma((r + 3) // 2)
                for u in range(4 * (r - 1), 4 * r):
                    if u < NUNITS:
                        gold_mm(u)

            # drain remaining gold units
            for u in range(4 * (L - 1), NUNITS):
                gold_mm(u)

            # ---------------- epilogue ----------------
            # bw final matmul: g_k = W h_k
            gfin = qb_ps.tile([128, 512], fp32, tag="qb")
            nc.tensor.matmul(gfin[:, 0:NW], wt_bf, bst, start=True, stop=True)
            # couplings: cpl[:, c] = f[:, c] * g[:, c]
            cpl = dumpp.tile([128, NW], bf16, tag="cpl")
            nc.vector.tensor_tensor(
                out=cpl, in0=gfin[:, 0:NW], in1=fst, op=ALU.mult,
            )
            csum_c = m_ps.tile([1, NW], fp32, tag="m")
            nc.tensor.matmul(csum_c, ones_col_bf, cpl, start=True, stop=True)
            csum_f = m_ps.tile([1, NW], fp32, tag="m")
            nc.tensor.matmul(csum_f, ones_col_bf, fst, start=True, stop=True)
            lncpl = smallp.tile([1, NW], fp32, tag="lncpl")
            acc_cpl = consts.tile([1, 1], fp32)
            nc.scalar.activation(lncpl, csum_c, AF.Ln, accum_out=acc_cpl)
            lnfs = smallp.tile([1, NW - BC], fp32, tag="lnfs")
            acc_fs = consts.tile([1, 1], fp32)
            nc.scalar.activation(lnfs, csum_f[:, BC:NW], AF.Ln, accum_out=acc_fs)
            lnglog = smallp.tile([1, NREN * (NW + BC)], fp32, tag="lnglog")
            acc_gl = consts.tile([1, 1], fp32)
            nc.scalar.activation(lnglog, glog, AF.Ln, accum_out=acc_gl)

            # gold finalize
            cdump = dumpp.tile([128, 256], fp32, tag="cdump")
            nc.scalar.activation(cdump, gold_ps[:, 0:256], AF.Copy)
            cmul = dumpp.tile([128, 256], fp32, tag="cmul")
            nc.vector.tensor_tensor(out=cmul, in0=cdump, in1=tri, op=ALU.mult)
            rowred = smallp.tile([128, 1], fp32, tag="rowred")
            nc.vector.tensor_reduce(rowred, cmul, axis=AX.X, op=ALU.add)
            goldtot_ps = m_ps.tile([1, 1], fp32, tag="m")
            nc.tensor.matmul(goldtot_ps, ones_col_f, rowred, start=True, stop=True)

            # loss_sum = acc_cpl - acc_fs + acc_gl + chat16 - goldtot
            res = smallp.tile([1, 1], fp32, tag="res")
            nc.vector.tensor_tensor(out=res, in0=acc_cpl, in1=acc_fs,
                                    op=ALU.subtract)
            nc.vector.tensor_tensor(out=res, in0=res, in1=acc_gl, op=ALU.add)
            nc.vector.tensor_tensor(out=res, in0=res, in1=chat16, op=ALU.add)
            nc.vector.tensor_tensor(out=res, in0=res, in1=goldtot_ps,
                                    op=ALU.subtract)
            nc.sync.dma_start(out=out_d[:], in_=res[0:1, :])

    return nc


def _get_compiled(finalized=False):
    global _compiled
    if _compiled is None:
        _compiled = _build_program()
    if finalized and not _compiled.is_finalized():
        _compiled.finalize()
    return _compiled


def make_in_maps(emissions, transitions, tags):
    bf = ml_dtypes.bfloat16
    f8 = ml_dtypes.float8_e4m3
    tr32 = np.ascontiguousarray(transitions, dtype=np.float32)
    # chat exactly as the device computes W: bf16(exp(trans)) colsums
    Wh = np.exp(tr32).astype(bf).astype(np.float32)
    chat = np.float32(np.log(Wh.sum(axis=0)[1:]).mean())
    chat_arr = np.array([chat], dtype=np.float32)
    jj = np.arange(T)
    in_maps = []
    for c in range(NCORES):
        sl = slice(c * BC, (c + 1) * BC)
        em = np.asarray(emissions[sl], dtype=np.float32)
        emc_bf = (em - chat).astype(bf)             # [16, 2048, 128]
        em4 = emc_bf.reshape(BC, K, L, T)           # [b, k, r, tag]
        efw = em4[:, 0:K - 1].transpose(3, 2, 1, 0)           # [tag, r, k, b]
        ebw = em4[:, 1:K, ::-1].transpose(3, 2, 1, 0)         # [tag, r, k-1, b]
        elay = np.zeros((T, L, BLK), dtype=bf)
        elay[:, :, 0:NW] = efw.reshape(T, L, NW)
        elay[:, :, NW + PAD:NW + PAD + NW] = ebw.reshape(T, L, NW)

        tg = np.asarray(tags[sl]).astype(np.int64)            # [16, 2048]
        tgsh = np.concatenate(
            [tg[:, 1:], np.full((BC, 1), JUNK_TAG, np.int64)], axis=1
        )
        em_f8 = np.asarray(emissions[sl], dtype=np.float32).astype(f8)
        # one-hots as fp8 {0,1}
        oh = (tg[:, :, None] == jj).astype(f8)                # [b, s, j]
        ohsh = (tgsh[:, :, None] == jj).astype(f8)
        # ring layout: [128(s) x ring x (oh oct | ohsh oct | emis oct)]
        def ringify(x):      # [b, s, j] -> [s128, ring, u, j]
            x6 = x.reshape(BC, NSB, 128, T)                   # [b, sb, s, j]
            x6 = x6.reshape(BC, NRINGS // BC, RU, 128, T)     # [b, rg, u, s, j]
            return x6.transpose(3, 0, 1, 2, 4).reshape(
                128, NRINGS, RU * T)                          # ring = b*2+rg
        glay = np.empty((128, NRINGS, RCOLS), dtype=f8)
        glay[:, :, 0:RU * T] = ringify(oh)
        glay[:, :, RU * T:2 * RU * T] = ringify(ohsh)
        glay[:, :, 2 * RU * T:] = ringify(em_f8)
        in_maps.append({
            "e_lay": np.ascontiguousarray(elay.reshape(T, L * BLK)),
            "gold_lay": np.ascontiguousarray(glay.reshape(128, NRINGS * RCOLS)),
            "transitions": tr32,
            "chat": chat_arr,
        })
    return in_maps


def _run_device(emissions, transitions, tags):
    from concourse.bass_utils import run_bass_kernel_spmd

    nc = _get_compiled(finalized=True)
    res = run_bass_kernel_spmd(
        nc, make_in_maps(emissions, transitions, tags), list(range(NCORES))
    )
    tot = sum(float(res.results[c]["loss_parts"][0]) for c in range(NCORES))
    return np.float32(tot / B)


def _run_host(emissions, transitions, tags, mask):
    """Slow but fully general fallback (any mask pattern)."""
    e = emissions.astype(np.float64)
    t = transitions.astype(np.float64)

    def lse(x, axis):
        m = x.max(axis=axis, keepdims=True)
        return (m + np.log(np.exp(x - m).sum(axis=axis, keepdims=True))).squeeze(axis)

    score = e[:, 0]
    for s in range(1, e.shape[1]):
        nxt = lse(score[:, :, None] + t[None, :, :] + e[:, s, None, :], axis=1)
        score = np.where(mask[:, s, None], nxt, score)
    log_Z = lse(score, axis=1)
    emit = np.take_along_axis(e, tags[..., None].astype(np.int64), axis=2)[..., 0]
    trans_sc = t[tags[:, :-1].astype(np.int64), tags[:, 1:].astype(np.int64)]
    m = mask[:, 1:].astype(np.float64)
    seq = emit[:, 0] + ((trans_sc + emit[:, 1:]) * m).sum(axis=1)
    return np.float32((log_Z - seq).mean())


def kernel(emissions, transitions, tags, mask):
    emissions = np.asarray(emissions)
    transitions = np.asarray(transitions)
    tags = np.asarray(tags)
    mask = np.asarray(mask)
    if emissions.shape != (B, S, T) or not mask.all():
        return _run_host(emissions, transitions, tags, mask)
    return _run_device(emissions, transitions, tags)


# revision 20
# speedup vs baseline: 6.3722x; 1.0610x over previous
"""Trainium2 Bass kernel for a batched linear-chain CRF negative log-likelihood.

reference semantics (B=128, S=2048, T=128):
    forward algorithm over S steps -> log_Z per batch
    gold path score = emissions gathered at tags + transitions gathered at
    (tag_t, tag_{t+1}) pairs, summed over time
    output = mean(log_Z - seq_score)   (scalar f32)

Strategy (v8): segmented rank-1 forward algorithm, K=64 segments.
  - data parallel over 8 cores: 16 batch rows per core, transitions replicated.
  - linear space: M_t = diag(E_t) W^T with W = exp(transitions),
    E_t = exp(emit_t - chat).  Z = 1^T M_{S-1}..M_1 a0,  a0 = E_0.
  - split S into K=64 segments of L=32.  Products of positive matrices
    contract to rank-1, so P_k ~ f_k g_k^T / s_k with f_k = P_k 1,
    g_k = P_k^T 1, s_k = 1^T P_k 1.  Then
      ln Z ~ sum_k ln(g_k . f_{k-1}) - sum_k ln s_k + parked renorm logs
    with segment 0 run on the true a0 and segment K-1 only backward.
    (rank-1 error validated < 0.1 absolute on lnZ ~ 12000; tol ~ 6600.)
  - 63 chains per direction (1008 state cols) advance together per
    rotation: two 504-col matmuls per direction (stationary W / W^T) into
    a 2-bank PSUM tile [504|pad|504|pad], then ONE DVE multiply per
    direction with a host-prelaid E slice updates that direction's state.
    31 rotations instead of 2047 sequential steps; PE stays warm.
  - gold path: per (b, sblock) one-hot count matmuls accumulate a single
    C|D PSUM region for the whole core (mean-only output).  One-hots are
    HOST-ENCODED as fp8 (exact 0/1) and streamed from HBM in 8-unit ring
    blocks [oh oct | ohsh oct | emis oct]; 8 unit-matmuls per rotation.
  - host prep: bf16 cast + chat subtraction + layout permutation for E,
    fp8 one-hot/emission encoding for gold.
"""

import numpy as np
import ml_dtypes

B, S, T = 128, 2048, 128
NCORES = 8
BC = B // NCORES        # 16 batch rows per core
L = 32                  # segment length (rotations)
K = S // L              # 64 segments
NCH = K - 1             # 63 chains per direction
NW = NCH * BC           # 1008 state columns per direction
SUB = 504               # per-matmul column group (one PSUM bank)
DBLK = 1024             # padded per-direction block [504|8|504|8]
BLK = 2 * DBLK          # 2048: per-rotation E block (fw | bw)
NSB = S // 128          # 16 s-blocks for gold
NUNITS = BC * NSB       # 256 gold units
RU = 8                  # gold units per ring
NRINGS = NUNITS // RU   # 32
RCOLS = 3 * RU * 128    # 3072: ring block cols [oh | ohsh | emis]
JUNK_TAG = 60000
REN_FW = (15,)
REN_BW = (15, 31)
NREN = 2                # glog slots for bw; fw uses 1

_compiled = None


def _build_program():
    import concourse.bass as bass
    import concourse.bacc as bacc
    import concourse.tile as tile
    from concourse import mybir
    from concourse.masks import make_identity

    fp32 = mybir.dt.float32
    bf16 = mybir.dt.bfloat16
    fp8 = mybir.dt.float8e4
    AF = mybir.ActivationFunctionType
    ALU = mybir.AluOpType
    AX = mybir.AxisListType

    nc = bacc.Bacc(None)
    e_d = nc.declare_dram_parameter("e_lay", [128, L * BLK], bf16, isOutput=False)
    g_d = nc.declare_dram_parameter("gold_lay", [128, NRINGS * RCOLS], fp8,
                                    isOutput=False)
    tr_d = nc.declare_dram_parameter("transitions", [T, T], fp32, isOutput=False)
    ch_d = nc.declare_dram_parameter("chat", [1], fp32, isOutput=False)
    out_d = nc.declare_dram_parameter("loss_parts", [1], fp32, isOutput=True)

    S0, S1 = 0, 512          # sub-block col offsets within a direction block
    W16 = 1016               # cols 0:1016 = [504 | pad8 | 504] active span

    with tile.TileContext(nc) as tc:
        with (
            tc.tile_pool(name="consts", bufs=1) as consts,
            tc.tile_pool(name="ebuf", bufs=1) as ebufp,
            tc.tile_pool(name="estg", bufs=3) as estgp,
            tc.tile_pool(name="state", bufs=3) as statep,
            tc.tile_pool(name="ring", bufs=3) as ringp,
            tc.tile_pool(name="small", bufs=1) as smallp,
            tc.tile_pool(name="dump", bufs=1) as dumpp,
            tc.tile_pool(name="qf_ps", bufs=1, space="PSUM") as qf_ps,
            tc.tile_pool(name="qb_ps", bufs=1, space="PSUM") as qb_ps,
            tc.tile_pool(name="g_ps", bufs=1, space="PSUM") as g_ps,
            tc.tile_pool(name="m_ps", bufs=1, space="PSUM") as m_ps,
        ):
            # ---------------- constants ----------------
            ident = consts.tile([128, 128], fp32)
            make_identity(nc, ident)
            ident_bf = consts.tile([128, 128], bf16)
            make_identity(nc, ident_bf)
            ones_col_bf = consts.tile([128, 1], bf16)
            nc.vector.memset(ones_col_bf, 1.0)
            ones_col_f = consts.tile([128, 1], fp32)
            nc.vector.memset(ones_col_f, 1.0)
            ones_row_bf = consts.tile([1, 128], bf16)
            nc.vector.memset(ones_row_bf, 1.0)

            tr_sb = consts.tile([128, 128], fp32)
            nc.sync.dma_start(out=tr_sb, in_=tr_d[:, :])
            chat_sb = consts.tile([1, 1], fp32)
            nc.sync.dma_start(out=chat_sb, in_=ch_d[:])

            w_bf = consts.tile([128, 128], bf16)
            nc.scalar.activation(w_bf, tr_sb, AF.Exp)
            wt_psum = m_ps.tile([128, 128], bf16, tag="m")
            nc.tensor.transpose(wt_psum, w_bf, ident_bf)
            wt_bf = consts.tile([128, 128], bf16)
            nc.vector.tensor_copy(wt_bf, wt_psum)

            # [trans | identity] for the gold finalize
            tri = consts.tile([128, 256], fp32)
            nc.vector.tensor_copy(tri[:, 0:128], tr_sb)
            nc.vector.tensor_copy(tri[:, 128:256], ident)

            # rho = W^T 1 (colsums of W) as [128,1]
            rho_ps = m_ps.tile([128, 1], fp32, tag="m")
            nc.tensor.matmul(rho_ps, w_bf, ones_col_bf, start=True, stop=True)
            rho = consts.tile([128, 1], fp32)
            nc.vector.tensor_copy(rho, rho_ps)
            # BC * S * chat correction
            chat16 = consts.tile([1, 1], fp32)
            nc.scalar.activation(chat16, chat_sb, AF.Copy,
                                 scale=float(BC) * float(S))

            # parked renorm logs: bw 2xNW | fw(k=0 only) 1xBC
            glog = consts.tile([1, NREN * NW + BC], fp32)
            nc.vector.memset(glog, 1.0)

            # ---------------- E supply (DMA + exp) ----------------
            ebuf = ebufp.tile([128, L * BLK], bf16)  # 128KB/partition

            def emit_echunk(c):
                stg = estgp.tile([128, BLK], bf16, tag="estg")
                nc.sync.dma_start(out=stg, in_=e_d[:, c * BLK:(c + 1) * BLK])
                nc.scalar.activation(ebuf[:, c * BLK:(c + 1) * BLK], stg, AF.Exp)

            for c in range(3):
                emit_echunk(c)

            # ---------------- gold machinery ----------------
            gold_ps = g_ps.tile([128, 512], fp32, tag="gold")
            rings = {}

            def gold_dma(ri):
                ring = ringp.tile([128, RCOLS], fp8, tag="ring", name=f"ring_{ri}")
                nc.sync.dma_start(out=ring, in_=g_d[:, ri * RCOLS:(ri + 1) * RCOLS])
                rings[ri] = ring

            def gold_mm(u):
                ri, q = u // RU, u % RU
                ring = rings[ri]
                mv = ring[:, RU * 128:].rearrange(
                    "p (h x) -> p h x", h=2)[:, :, q * 128:(q + 1) * 128]
                nc.tensor.matmul(
                    gold_ps[:, 0:256], ring[:, q * 128:(q + 1) * 128], mv,
                    start=(u == 0), stop=(u == NUNITS - 1),
                )
                if q == RU - 1:
                    rings.pop(ri)

            gold_dma(0)
            gold_dma(1)

            # ---------------- chain states ----------------
            # fst cols: chain c = k*16+b (k=0..62) at physical col c + 8*(c>=504)
            # bst: chain (k-1)*16+b covers seg k (k=1..63), same padding
            fst = statep.tile([128, DBLK], bf16, tag="fst")
            nc.vector.tensor_copy(fst[:, 0:BC], ebuf[:, 0:BC])
            nc.vector.tensor_scalar(
                out=fst[:, BC:SUB], in0=ebuf[:, BC:SUB], scalar1=rho,
                scalar2=None, op0=ALU.mult,
            )
            nc.vector.tensor_scalar(
                out=fst[:, S1:W16], in0=ebuf[:, S1:W16], scalar1=rho,
                scalar2=None, op0=ALU.mult,
            )
            bst = statep.tile([128, DBLK], bf16, tag="bst")
            nc.vector.tensor_copy(bst[:, 0:W16], ebuf[:, DBLK:DBLK + W16])

            def renorm(state, parks, tag):
                # parks: list of (cs_lo, cs_hi, glog_col) column ranges to park
                cs = m_ps.tile([1, DBLK], fp32, tag="m")
                nc.tensor.matmul(cs[:, 0:SUB], ones_col_bf, state[:, 0:SUB],
                                 start=True, stop=True)
                nc.tensor.matmul(cs[:, S1:W16], ones_col_bf, state[:, S1:W16],
                                 start=True, stop=True)
                for lo, hi, gc in parks:
                    nc.vector.tensor_copy(glog[:, gc:gc + hi - lo], cs[:, lo:hi])
                rec = smallp.tile([1, DBLK], fp32, tag="rec")
                nc.vector.reciprocal_approx_fast(out=rec[:, 0:W16],
                                                 in_=cs[:, 0:W16])
                recb = smallp.tile([1, DBLK], bf16, tag="recb")
                nc.vector.tensor_copy(recb[:, 0:W16], rec[:, 0:W16])
                bc_ps = m_ps.tile([128, DBLK], fp32, tag="m")
                nc.tensor.matmul(bc_ps[:, 0:SUB], ones_row_bf, recb[:, 0:SUB],
                                 start=True, stop=True)
                nc.tensor.matmul(bc_ps[:, S1:W16], ones_row_bf, recb[:, S1:W16],
                                 start=True, stop=True)
                out = statep.tile([128, DBLK], bf16, tag=tag)
                nc.vector.tensor_tensor(out=out[:, 0:W16], in0=state[:, 0:W16],
                                        in1=bc_ps[:, 0:W16], op=ALU.mult)
                return out

            # ---------------- rotation loop ----------------
            nren = 0
            for r in range(1, L):
                qf = qf_ps.tile([128, DBLK], fp32, tag="qf")
                nc.tensor.matmul(qf[:, 0:SUB], w_bf, fst[:, 0:SUB],
                                 start=True, stop=True)
                nc.tensor.matmul(qf[:, S1:W16], w_bf, fst[:, S1:W16],
                                 start=True, stop=True)
                qb = qb_ps.tile([128, DBLK], fp32, tag="qb")
                nc.tensor.matmul(qb[:, 0:SUB], wt_bf, bst[:, 0:SUB],
                                 start=True, stop=True)
                nc.tensor.matmul(qb[:, S1:W16], wt_bf, bst[:, S1:W16],
                                 start=True, stop=True)
                nfst = statep.tile([128, DBLK], bf16, tag="fst")
                nc.vector.tensor_tensor(
                    out=nfst[:, 0:W16], in0=qf[:, 0:W16],
                    in1=ebuf[:, r * BLK:r * BLK + W16], op=ALU.mult,
                )
                fst = nfst
                nbst = statep.tile([128, DBLK], bf16, tag="bst")
                nc.vector.tensor_tensor(
                    out=nbst[:, 0:W16], in0=qb[:, 0:W16],
                    in1=ebuf[:, r * BLK + DBLK:r * BLK + DBLK + W16], op=ALU.mult,
                )
                bst = nbst
                if r in REN_FW:
                    fst = renorm(fst, [(0, BC, NREN * NW)], "fst")
                if r in REN_BW:
                    bst = renorm(
                        bst,
                        [(0, SUB, nren * NW), (S1, W16, nren * NW + SUB)],
                        "bst",
                    )
                    nren += 1
                # E prefetch (two chunks ahead)
                if r + 2 < L:
                    emit_echunk(r + 2)
                # gold: 8 unit-mms per rotation (ring r-1); DMA ring r+1
                if r + 1 < NRINGS:
                    gold_dma(r + 1)
                for u in range(RU * (r - 1), RU * r):
                    gold_mm(u)

            # drain ring 31
            for u in range(RU * (L - 1), NUNITS):
                gold_mm(u)

            # ---------------- epilogue ----------------
            # bw final matmul: g_k = W h_k
            gfin = qb_ps.tile([128, DBLK], fp32, tag="qb")
            nc.tensor.matmul(gfin[:, 0:SUB], wt_bf, bst[:, 0:SUB],
                             start=True, stop=True)
            nc.tensor.matmul(gfin[:, S1:W16], wt_bf, bst[:, S1:W16],
                             start=True, stop=True)
            # couplings: cpl[:, c] = f[:, c] * g[:, c]
            cpl = dumpp.tile([128, DBLK], bf16, tag="cpl")
            nc.vector.tensor_tensor(
                out=cpl[:, 0:W16], in0=gfin[:, 0:W16], in1=fst[:, 0:W16],
                op=ALU.mult,
            )
            csum_c = m_ps.tile([1, DBLK], fp32, tag="m")
            nc.tensor.matmul(csum_c[:, 0:SUB], ones_col_bf, cpl[:, 0:SUB],
                             start=True, stop=True)
            nc.tensor.matmul(csum_c[:, S1:W16], ones_col_bf, cpl[:, S1:W16],
                             start=True, stop=True)
            lncpl = smallp.tile([1, DBLK], fp32, tag="lncpl")
            acc_c1 = consts.tile([1, 1], fp32)
            nc.scalar.activation(lncpl[:, 0:SUB], csum_c[:, 0:SUB], AF.Ln,
                                 accum_out=acc_c1)
            acc_c2 = consts.tile([1, 1], fp32)
            nc.scalar.activation(lncpl[:, S1:W16], csum_c[:, S1:W16], AF.Ln,
                                 accum_out=acc_c2)
            csum_f = m_ps.tile([1, DBLK], fp32, tag="m")
            nc.tensor.matmul(csum_f[:, 0:SUB], ones_col_bf, fst[:, 0:SUB],
                             start=True, stop=True)
            nc.tensor.matmul(csum_f[:, S1:W16], ones_col_bf, fst[:, S1:W16],
                             start=True, stop=True)
            lnfs = smallp.tile([1, DBLK], fp32, tag="lnfs")
            acc_f1 = consts.tile([1, 1], fp32)
            nc.scalar.activation(lnfs[:, BC:SUB], csum_f[:, BC:SUB], AF.Ln,
                                 accum_out=acc_f1)
            acc_f2 = consts.tile([1, 1], fp32)
            nc.scalar.activation(lnfs[:, S1:W16], csum_f[:, S1:W16], AF.Ln,
                                 accum_out=acc_f2)
            lnglog = smallp.tile([1, NREN * NW + BC], fp32, tag="lnglog")
            acc_gl = consts.tile([1, 1], fp32)
            nc.scalar.activation(lnglog, glog, AF.Ln, accum_out=acc_gl)

            # gold finalize
            cdump = dumpp.tile([128, 256], fp32, tag="cdump")
            nc.scalar.activation(cdump, gold_ps[:, 0:256], AF.Copy)
            cmul = dumpp.tile([128, 256], fp32, tag="cmul")
            nc.vector.tensor_tensor(out=cmul, in0=cdump, in1=tri, op=ALU.mult)
            rowred = smallp.tile([128, 1], fp32, tag="rowred")
            nc.vector.tensor_reduce(rowred, cmul, axis=AX.X, op=ALU.add)
            goldtot_ps = m_ps.tile([1, 1], fp32, tag="m")
            nc.tensor.matmul(goldtot_ps, ones_col_f, rowred, start=True, stop=True)

            # loss_sum = acc_c1+acc_c2 - acc_f1-acc_f2 + acc_gl + chat16 - goldtot
            res = smallp.tile([1, 1], fp32, tag="res")
            nc.vector.tensor_tensor(out=res, in0=acc_c1, in1=acc_c2, op=ALU.add)
            nc.vector.tensor_tensor(out=res, in0=res, in1=acc_f1, op=ALU.subtract)
            nc.vector.tensor_tensor(out=res, in0=res, in1=acc_f2, op=ALU.subtract)
            nc.vector.tensor_tensor(out=res, in0=res, in1=acc_gl, op=ALU.add)
            nc.vector.tensor_tensor(out=res, in0=res, in1=chat16, op=ALU.add)
            nc.vector.tensor_tensor(out=res, in0=res, in1=goldtot_ps,
                                    op=ALU.subtract)
            nc.sync.dma_start(out=out_d[:], in_=res[0:1, :])

    return nc


def _get_compiled(finalized=False):
    global _compiled
    if _compiled is None:
        _compiled = _build_program()
    if finalized and not _compiled.is_finalized():
        _compiled.finalize()
    return _compiled


def _pad_cols(x):
    """[..., 1008] -> [..., 1024] with pads at 504:512 and 1016:1024."""
    out = np.zeros(x.shape[:-1] + (DBLK,), dtype=x.dtype)
    out[..., 0:SUB] = x[..., 0:SUB]
    out[..., 512:1016] = x[..., SUB:NW]
    return out


SUB_, NW_ = 504, 1008


def make_in_maps(emissions, transitions, tags):
    bf = ml_dtypes.bfloat16
    f8 = ml_dtypes.float8_e4m3
    tr32 = np.ascontiguousarray(transitions, dtype=np.float32)
    Wh = np.exp(tr32).astype(bf).astype(np.float32)
    chat = np.float32(np.log(Wh.sum(axis=0)[1:]).mean())
    chat_arr = np.array([chat], dtype=np.float32)
    jj = np.arange(T)
    in_maps = []
    for c in range(NCORES):
        sl = slice(c * BC, (c + 1) * BC)
        em = np.asarray(emissions[sl], dtype=np.float32)
        emc_bf = (em - chat).astype(bf)             # [16, 2048, 128]
        em4 = emc_bf.reshape(BC, K, L, T)           # [b, k, r, tag]
        efw = em4[:, 0:K - 1].transpose(3, 2, 1, 0).reshape(T, L, NW)
        ebw = em4[:, 1:K, ::-1].transpose(3, 2, 1, 0).reshape(T, L, NW)
        elay = np.zeros((T, L, BLK), dtype=bf)
        elay[:, :, 0:DBLK] = _pad_cols(efw)
        elay[:, :, DBLK:BLK] = _pad_cols(ebw)

        tg = np.asarray(tags[sl]).astype(np.int64)            # [16, 2048]
        tgsh = np.concatenate(
            [tg[:, 1:], np.full((BC, 1), JUNK_TAG, np.int64)], axis=1
        )
        em_f8 = np.asarray(emissions[sl], dtype=np.float32).astype(f8)
        oh = (tg[:, :, None] == jj).astype(f8)                # [b, s, j]
        ohsh = (tgsh[:, :, None] == jj).astype(f8)

        def ringify(x):      # [b, s, j] -> [s128, ring, u, j]
            x6 = x.reshape(BC, NRINGS // BC, RU, 128, T)      # [b, rg, u, s, j]
            return x6.transpose(3, 0, 1, 2, 4).reshape(128, NRINGS, RU * T)

        glay = np.empty((128, NRINGS, RCOLS), dtype=f8)
        glay[:, :, 0:RU * T] = ringify(oh)
        glay[:, :, RU * T:2 * RU * T] = ringify(ohsh)
        glay[:, :, 2 * RU * T:] = ringify(em_f8)
        in_maps.append({
            "e_lay": np.ascontiguousarray(elay.reshape(T, L * BLK)),
            "gold_lay": np.ascontiguousarray(glay.reshape(128, NRINGS * RCOLS)),
            "transitions": tr32,
            "chat": chat_arr,
        })
    return in_maps


def _run_device(emissions, transitions, tags):
    from concourse.bass_utils import run_bass_kernel_spmd

    nc = _get_compiled(finalized=True)
    res = run_bass_kernel_spmd(
        nc, make_in_maps(emissions, transitions, tags), list(range(NCORES))
    )
    tot = sum(float(res.results[c]["loss_parts"][0]) for c in range(NCORES))
    return np.float32(tot / B)


def _run_host(emissions, transitions, tags, mask):
    """Slow but fully general fallback (any mask pattern)."""
    e = emissions.astype(np.float64)
    t = transitions.astype(np.float64)

    def lse(x, axis):
        m = x.max(axis=axis, keepdims=True)
        return (m + np.log(np.exp(x - m).sum(axis=axis, keepdims=True))).squeeze(axis)

    score = e[:, 0]
    for s in range(1, e.shape[1]):
        nxt = lse(score[:, :, None] + t[None, :, :] + e[:, s, None, :], axis=1)
        score = np.where(mask[:, s, None], nxt, score)
    log_Z = lse(score, axis=1)
    emit = np.take_along_axis(e, tags[..., None].astype(np.int64), axis=2)[..., 0]
    trans_sc = t[tags[:, :-1].astype(np.int64), tags[:, 1:].astype(np.int64)]
    m = mask[:, 1:].astype(np.float64)
    seq = emit[:, 0] + ((trans_sc + emit[:, 1:]) * m).sum(axis=1)
    return np.float32((log_Z - seq).mean())


def kernel(emissions, transitions, tags, mask):
    emissions = np.asarray(emissions)
    transitions = np.asarray(transitions)
    tags = np.asarray(tags)
    mask = np.asarray(mask)
    if emissions.shape != (B, S, T) or not mask.all():
        return _run_host(emissions, transitions, tags, mask)
    return _run_device(emissions, transitions, tags)


# revision 23
# speedup vs baseline: 6.6785x; 1.0481x over previous
"""Trainium2 Bass kernel for a batched linear-chain CRF negative log-likelihood.

reference semantics (B=128, S=2048, T=128):
    forward algorithm over S steps -> log_Z per batch
    gold path score = emissions gathered at tags + transitions gathered at
    (tag_t, tag_{t+1}) pairs, summed over time
    output = mean(log_Z - seq_score)   (scalar f32)

Strategy (v8): segmented rank-1 forward algorithm, K=64 segments.
  - data parallel over 8 cores: 16 batch rows per core, transitions replicated.
  - linear space: M_t = diag(E_t) W^T with W = exp(transitions),
    E_t = exp(emit_t - chat).  Z = 1^T M_{S-1}..M_1 a0,  a0 = E_0.
  - split S into K=64 segments of L=32.  Products of positive matrices
    contract to rank-1, so P_k ~ f_k g_k^T / s_k with f_k = P_k 1,
    g_k = P_k^T 1, s_k = 1^T P_k 1.  Then
      ln Z ~ sum_k ln(g_k . f_{k-1}) - sum_k ln s_k + parked renorm logs
    with segment 0 run on the true a0 and segment K-1 only backward.
    (rank-1 error validated < 0.1 absolute on lnZ ~ 12000; tol ~ 6600.)
  - 63 chains per direction (1008 state cols) advance together per
    rotation: two 504-col matmuls per direction (stationary W / W^T) into
    a 2-bank PSUM tile [504|pad|504|pad], then ONE DVE multiply per
    direction with a host-prelaid E slice updates that direction's state.
    31 rotations instead of 2047 sequential steps; PE stays warm.
  - gold path: per (b, sblock) one-hot count matmuls accumulate a single
    C|D PSUM region for the whole core (mean-only output).  One-hots are
    HOST-ENCODED as fp8 (exact 0/1) and streamed from HBM in 8-unit ring
    blocks [oh oct | ohsh oct | emis oct]; 8 unit-matmuls per rotation.
  - host prep: bf16 cast + chat subtraction + layout permutation for E,
    fp8 one-hot/emission encoding for gold.
"""

import numpy as np
import ml_dtypes

B, S, T = 128, 2048, 128
NCORES = 8
BC = B // NCORES        # 16 batch rows per core
L = 32                  # segment length (rotations)
K = S // L              # 64 segments
NCH = K - 1             # 63 chains per direction
NW = NCH * BC           # 1008 state columns per direction
SUB = 504               # per-matmul column group (one PSUM bank)
DBLK = 1024             # padded per-direction block [504|8|504|8]
BLK = 2 * DBLK          # 2048: per-rotation E block (fw | bw)
NSB = S // 128          # 16 s-blocks for gold
NUNITS = BC * NSB       # 256 gold units
RU = 8                  # gold units per ring
NRINGS = NUNITS // RU   # 32
RCOLS = 3 * RU * 128    # 3072: ring block cols [oh | ohsh | emis]
JUNK_TAG = 60000
REN_FW = (15,)
REN_BW = (15, 31)
NREN = 2                # glog slots for bw; fw uses 1

_compiled = None


def _build_program():
    import concourse.bass as bass
    import concourse.bacc as bacc
    import concourse.tile as tile
    from concourse import mybir
    from concourse.masks import make_identity

    fp32 = mybir.dt.float32
    bf16 = mybir.dt.bfloat16
    fp8 = mybir.dt.float8e4
    AF = mybir.ActivationFunctionType
    ALU = mybir.AluOpType
    AX = mybir.AxisListType

    nc = bacc.Bacc(None)
    e_d = nc.declare_dram_parameter("e_lay", [128, L * BLK], bf16, isOutput=False)
    g_d = nc.declare_dram_parameter("gold_lay", [128, NRINGS * RCOLS], fp8,
                                    isOutput=False)
    tr_d = nc.declare_dram_parameter("transitions", [T, T], fp32, isOutput=False)
    ch_d = nc.declare_dram_parameter("chat", [1], fp32, isOutput=False)
    out_d = nc.declare_dram_parameter("loss_parts", [1], fp32, isOutput=True)

    S0, S1 = 0, 512          # sub-block col offsets within a direction block
    W16 = 1016               # cols 0:1016 = [504 | pad8 | 504] active span

    with tile.TileContext(nc) as tc:
        with (
            tc.tile_pool(name="consts", bufs=1) as consts,
            tc.tile_pool(name="ebuf", bufs=1) as ebufp,
            tc.tile_pool(name="state", bufs=3) as statep,
            tc.tile_pool(name="ring", bufs=3) as ringp,
            tc.tile_pool(name="small", bufs=1) as smallp,
            tc.tile_pool(name="dump", bufs=1) as dumpp,
            tc.tile_pool(name="qf_ps", bufs=1, space="PSUM") as qf_ps,
            tc.tile_pool(name="qb_ps", bufs=1, space="PSUM") as qb_ps,
            tc.tile_pool(name="g_ps", bufs=1, space="PSUM") as g_ps,
            tc.tile_pool(name="m_ps", bufs=1, space="PSUM") as m_ps,
        ):
            # ---------------- constants ----------------
            ident = consts.tile([128, 128], fp32)
            make_identity(nc, ident)
            ident_bf = consts.tile([128, 128], bf16)
            make_identity(nc, ident_bf)
            ones_col_bf = consts.tile([128, 1], bf16)
            nc.vector.memset(ones_col_bf, 1.0)
            ones_col_f = consts.tile([128, 1], fp32)
            nc.vector.memset(ones_col_f, 1.0)
            ones_row_bf = consts.tile([1, 128], bf16)
            nc.vector.memset(ones_row_bf, 1.0)

            tr_sb = consts.tile([128, 128], fp32)
            nc.sync.dma_start(out=tr_sb, in_=tr_d[:, :])
            chat_sb = consts.tile([1, 1], fp32)
            nc.sync.dma_start(out=chat_sb, in_=ch_d[:])

            w_bf = consts.tile([128, 128], bf16)
            nc.scalar.activation(w_bf, tr_sb, AF.Exp)
            wt_psum = m_ps.tile([128, 128], bf16, tag="m")
            nc.tensor.transpose(wt_psum, w_bf, ident_bf)
            wt_bf = consts.tile([128, 128], bf16)
            nc.vector.tensor_copy(wt_bf, wt_psum)

            # [trans | identity] for the gold finalize
            tri = consts.tile([128, 256], fp32)
            nc.vector.tensor_copy(tri[:, 0:128], tr_sb)
            nc.vector.tensor_copy(tri[:, 128:256], ident)

            # rho = W^T 1 (colsums of W) as [128,1]
            rho_ps = m_ps.tile([128, 1], fp32, tag="m")
            nc.tensor.matmul(rho_ps, w_bf, ones_col_bf, start=True, stop=True)
            rho = consts.tile([128, 1], fp32)
            nc.vector.tensor_copy(rho, rho_ps)
            # BC * S * chat correction
            chat16 = consts.tile([1, 1], fp32)
            nc.scalar.activation(chat16, chat_sb, AF.Copy,
                                 scale=float(BC) * float(S))

            # parked renorm logs: bw 2xNW | fw(k=0 only) 1xBC
            glog = consts.tile([1, NREN * NW + BC], fp32)
            nc.vector.memset(glog, 1.0)

            # ---------------- E supply (host-exp'd fp8, DMA only) -------
            ebuf = ebufp.tile([128, L * BLK], bf16)  # 128KB/partition

            def emit_echunk(c):
                nc.sync.dma_start(out=ebuf[:, c * BLK:(c + 1) * BLK],
                                  in_=e_d[:, c * BLK:(c + 1) * BLK])

            for c in range(3):
                emit_echunk(c)

            # ---------------- gold machinery ----------------
            gold_ps = g_ps.tile([128, 512], fp32, tag="gold")
            rings = {}

            def gold_dma(ri):
                ring = ringp.tile([128, RCOLS], fp8, tag="ring", name=f"ring_{ri}")
                nc.sync.dma_start(out=ring, in_=g_d[:, ri * RCOLS:(ri + 1) * RCOLS])
                rings[ri] = ring

            def gold_mm(u):
                ri, q = u // RU, u % RU
                ring = rings[ri]
                mv = ring[:, RU * 128:].rearrange(
                    "p (h x) -> p h x", h=2)[:, :, q * 128:(q + 1) * 128]
                nc.tensor.matmul(
                    gold_ps[:, 0:256], ring[:, q * 128:(q + 1) * 128], mv,
                    start=(u == 0), stop=(u == NUNITS - 1),
                )
                if q == RU - 1:
                    rings.pop(ri)

            gold_dma(0)
            gold_dma(1)

            # ---------------- chain states ----------------
            # fst cols: chain c = k*16+b (k=0..62) at physical col c + 8*(c>=504)
            # bst: chain (k-1)*16+b covers seg k (k=1..63), same padding
            fst = statep.tile([128, DBLK], bf16, tag="fst")
            nc.vector.tensor_copy(fst[:, 0:BC], ebuf[:, 0:BC])
            nc.vector.tensor_scalar(
                out=fst[:, BC:SUB], in0=ebuf[:, BC:SUB], scalar1=rho,
                scalar2=None, op0=ALU.mult,
            )
            nc.vector.tensor_scalar(
                out=fst[:, S1:W16], in0=ebuf[:, S1:W16], scalar1=rho,
                scalar2=None, op0=ALU.mult,
            )
            bst = statep.tile([128, DBLK], bf16, tag="bst")
            nc.vector.tensor_copy(bst[:, 0:W16], ebuf[:, DBLK:DBLK + W16])

            def renorm(state, parks, tag):
                # parks: list of (cs_lo, cs_hi, glog_col) column ranges to park
                cs = m_ps.tile([1, DBLK], fp32, tag="m")
                nc.tensor.matmul(cs[:, 0:SUB], ones_col_bf, state[:, 0:SUB],
                                 start=True, stop=True)
                nc.tensor.matmul(cs[:, S1:W16], ones_col_bf, state[:, S1:W16],
                                 start=True, stop=True)
                for lo, hi, gc in parks:
                    nc.vector.tensor_copy(glog[:, gc:gc + hi - lo], cs[:, lo:hi])
                rec = smallp.tile([1, DBLK], fp32, tag="rec")
                nc.vector.reciprocal_approx_fast(out=rec[:, 0:W16],
                                                 in_=cs[:, 0:W16])
                recb = smallp.tile([1, DBLK], bf16, tag="recb")
                nc.vector.tensor_copy(recb[:, 0:W16], rec[:, 0:W16])
                bc_ps = m_ps.tile([128, DBLK], fp32, tag="m")
                nc.tensor.matmul(bc_ps[:, 0:SUB], ones_row_bf, recb[:, 0:SUB],
                                 start=True, stop=True)
                nc.tensor.matmul(bc_ps[:, S1:W16], ones_row_bf, recb[:, S1:W16],
                                 start=True, stop=True)
                out = statep.tile([128, DBLK], bf16, tag=tag)
                nc.vector.tensor_tensor(out=out[:, 0:W16], in0=state[:, 0:W16],
                                        in1=bc_ps[:, 0:W16], op=ALU.mult)
                return out

            # ---------------- rotation loop ----------------
            nren = 0
            for r in range(1, L):
                qf = qf_ps.tile([128, DBLK], fp32, tag="qf")
                nc.tensor.matmul(qf[:, 0:SUB], w_bf, fst[:, 0:SUB],
                                 start=True, stop=True)
                nc.tensor.matmul(qf[:, S1:W16], w_bf, fst[:, S1:W16],
                                 start=True, stop=True)
                qb = qb_ps.tile([128, DBLK], fp32, tag="qb")
                nc.tensor.matmul(qb[:, 0:SUB], wt_bf, bst[:, 0:SUB],
                                 start=True, stop=True)
                nc.tensor.matmul(qb[:, S1:W16], wt_bf, bst[:, S1:W16],
                                 start=True, stop=True)
                nfst = statep.tile([128, DBLK], bf16, tag="fst")
                nc.vector.tensor_tensor(
                    out=nfst[:, 0:W16], in0=qf[:, 0:W16],
                    in1=ebuf[:, r * BLK:r * BLK + W16], op=ALU.mult,
                )
                fst = nfst
                nbst = statep.tile([128, DBLK], bf16, tag="bst")
                nc.vector.tensor_tensor(
                    out=nbst[:, 0:W16], in0=qb[:, 0:W16],
                    in1=ebuf[:, r * BLK + DBLK:r * BLK + DBLK + W16], op=ALU.mult,
                )
                bst = nbst
                if r in REN_FW:
                    fst = renorm(fst, [(0, BC, NREN * NW)], "fst")
                if r in REN_BW:
                    bst = renorm(
                        bst,
                        [(0, SUB, nren * NW), (S1, W16, nren * NW + SUB)],
                        "bst",
                    )
                    nren += 1
                # E prefetch (two chunks ahead)
                if r + 2 < L:
                    emit_echunk(r + 2)
                # gold: 8 unit-mms per rotation (ring r-1); DMA ring r+1
                if r + 1 < NRINGS:
                    gold_dma(r + 1)
                for u in range(RU * (r - 1), RU * r):
                    gold_mm(u)

            # drain ring 31
            for u in range(RU * (L - 1), NUNITS):
                gold_mm(u)

            # ---------------- epilogue ----------------
            # bw final matmul: g_k = W h_k
            gfin = qb_ps.tile([128, DBLK], fp32, tag="qb")
            nc.tensor.matmul(gfin[:, 0:SUB], wt_bf, bst[:, 0:SUB],
                             start=True, stop=True)
            nc.tensor.matmul(gfin[:, S1:W16], wt_bf, bst[:, S1:W16],
                             start=True, stop=True)
            # couplings: cpl[:, c] = f[:, c] * g[:, c]
            cpl = dumpp.tile([128, DBLK], bf16, tag="cpl")
            nc.vector.tensor_tensor(
                out=cpl[:, 0:W16], in0=gfin[:, 0:W16], in1=fst[:, 0:W16],
                op=ALU.mult,
            )
            csum_c = m_ps.tile([1, DBLK], fp32, tag="m")
            nc.tensor.matmul(csum_c[:, 0:SUB], ones_col_bf, cpl[:, 0:SUB],
                             start=True, stop=True)
            nc.tensor.matmul(csum_c[:, S1:W16], ones_col_bf, cpl[:, S1:W16],
                             start=True, stop=True)
            lncpl = smallp.tile([1, DBLK], fp32, tag="lncpl")
            acc_c1 = consts.tile([1, 1], fp32)
            nc.scalar.activation(lncpl[:, 0:SUB], csum_c[:, 0:SUB], AF.Ln,
                                 accum_out=acc_c1)
            acc_c2 = consts.tile([1, 1], fp32)
            nc.scalar.activation(lncpl[:, S1:W16], csum_c[:, S1:W16], AF.Ln,
                                 accum_out=acc_c2)
            csum_f = m_ps.tile([1, DBLK], fp32, tag="m")
            nc.tensor.matmul(csum_f[:, 0:SUB], ones_col_bf, fst[:, 0:SUB],
                             start=True, stop=True)
            nc.tensor.matmul(csum_f[:, S1:W16], ones_col_bf, fst[:, S1:W16],
                             start=True, stop=True)
            lnfs = smallp.tile([1, DBLK], fp32, tag="lnfs")
            acc_f1 = consts.tile([1, 1], fp32)
            nc.scalar.activation(lnfs[:, BC:SUB], csum_f[:, BC:SUB], AF.Ln,
                                 accum_out=acc_f1)
            acc_f2 = consts.tile([1, 1], fp32)
            nc.scalar.activation(lnfs[:, S1:W16], csum_f[:, S1:W16], AF.Ln,
                                 accum_out=acc_f2)
            lnglog = smallp.tile([1, NREN * NW + BC], fp32, tag="lnglog")
            acc_gl = consts.tile([1, 1], fp32)
            nc.scalar.activation(lnglog, glog, AF.Ln, accum_out=acc_gl)

            # gold finalize
            cdump = dumpp.tile([128, 256], fp32, tag="cdump")
            nc.scalar.activation(cdump, gold_ps[:, 0:256], AF.Copy)
            cmul = dumpp.tile([128, 256], fp32, tag="cmul")
            nc.vector.tensor_tensor(out=cmul, in0=cdump, in1=tri, op=ALU.mult)
            rowred = smallp.tile([128, 1], fp32, tag="rowred")
            nc.vector.tensor_reduce(rowred, cmul, axis=AX.X, op=ALU.add)
            goldtot_ps = m_ps.tile([1, 1], fp32, tag="m")
            nc.tensor.matmul(goldtot_ps, ones_col_f, rowred, start=True, stop=True)

            # loss_sum = acc_c1+acc_c2 - acc_f1-acc_f2 + acc_gl + chat16 - goldtot
            res = smallp.tile([1, 1], fp32, tag="res")
            nc.vector.tensor_tensor(out=res, in0=acc_c1, in1=acc_c2, op=ALU.add)
            nc.vector.tensor_tensor(out=res, in0=res, in1=acc_f1, op=ALU.subtract)
            nc.vector.tensor_tensor(out=res, in0=res, in1=acc_f2, op=ALU.subtract)
            nc.vector.tensor_tensor(out=res, in0=res, in1=acc_gl, op=ALU.add)
            nc.vector.tensor_tensor(out=res, in0=res, in1=chat16, op=ALU.add)
            nc.vector.tensor_tensor(out=res, in0=res, in1=goldtot_ps,
                                    op=ALU.subtract)
            nc.sync.dma_start(out=out_d[:], in_=res[0:1, :])

    return nc


def _get_compiled(finalized=False):
    global _compiled
    if _compiled is None:
        _compiled = _build_program()
    if finalized and not _compiled.is_finalized():
        _compiled.finalize()
    return _compiled


def _pad_cols(x):
    """[..., 1008] -> [..., 1024] with pads at 504:512 and 1016:1024."""
    out = np.zeros(x.shape[:-1] + (DBLK,), dtype=x.dtype)
    out[..., 0:SUB] = x[..., 0:SUB]
    out[..., 512:1016] = x[..., SUB:NW]
    return out


SUB_, NW_ = 504, 1008


def make_in_maps(emissions, transitions, tags):
    bf = ml_dtypes.bfloat16
    f8 = ml_dtypes.float8_e4m3
    tr32 = np.ascontiguousarray(transitions, dtype=np.float32)
    Wh = np.exp(tr32).astype(bf).astype(np.float32)
    chat = np.float32(np.log(Wh.sum(axis=0)[1:]).mean())
    chat_arr = np.array([chat], dtype=np.float32)
    jj = np.arange(T)
    in_maps = []
    for c in range(NCORES):
        sl = slice(c * BC, (c + 1) * BC)
        em = np.asarray(emissions[sl], dtype=np.float32)
        # E' = exp(em - (chat-3)), host-computed; bf16
        emc = np.exp(em - chat).astype(bf)
        em4 = emc.reshape(BC, K, L, T)              # [b, k, r, tag]
        efw = em4[:, 0:K - 1].transpose(3, 2, 1, 0).reshape(T, L, NW)
        ebw = em4[:, 1:K, ::-1].transpose(3, 2, 1, 0).reshape(T, L, NW)
        elay = np.zeros((T, L, BLK), dtype=bf)
        elay[:, :, 0:DBLK] = _pad_cols(efw)
        elay[:, :, DBLK:BLK] = _pad_cols(ebw)

        tg = np.asarray(tags[sl]).astype(np.int64)            # [16, 2048]
        tgsh = np.concatenate(
            [tg[:, 1:], np.full((BC, 1), JUNK_TAG, np.int64)], axis=1
        )
        em_f8 = np.asarray(emissions[sl], dtype=np.float32).astype(f8)
        oh = (tg[:, :, None] == jj).astype(f8)                # [b, s, j]
        ohsh = (tgsh[:, :, None] == jj).astype(f8)

        def ringify(x):      # [b, s, j] -> [s128, ring, u, j]
            x6 = x.reshape(BC, NRINGS // BC, RU, 128, T)      # [b, rg, u, s, j]
            return x6.transpose(3, 0, 1, 2, 4).reshape(128, NRINGS, RU * T)

        glay = np.empty((128, NRINGS, RCOLS), dtype=f8)
        glay[:, :, 0:RU * T] = ringify(oh)
        glay[:, :, RU * T:2 * RU * T] = ringify(ohsh)
        glay[:, :, 2 * RU * T:] = ringify(em_f8)
        in_maps.append({
            "e_lay": np.ascontiguousarray(elay.reshape(T, L * BLK)),
            "gold_lay": np.ascontiguousarray(glay.reshape(128, NRINGS * RCOLS)),
            "transitions": tr32,
            "chat": chat_arr,
        })
    return in_maps


def _run_device(emissions, transitions, tags):
    from concourse.bass_utils import run_bass_kernel_spmd

    nc = _get_compiled(finalized=True)
    res = run_bass_kernel_spmd(
        nc, make_in_maps(emissions, transitions, tags), list(range(NCORES))
    )
    tot = sum(float(res.results[c]["loss_parts"][0]) for c in range(NCORES))
    return np.float32(tot / B)


def _run_host(emissions, transitions, tags, mask):
    """Slow but fully general fallback (any mask pattern)."""
    e = emissions.astype(np.float64)
    t = transitions.astype(np.float64)

    def lse(x, axis):
        m = x.max(axis=axis, keepdims=True)
        return (m + np.log(np.exp(x - m).sum(axis=axis, keepdims=True))).squeeze(axis)

    score = e[:, 0]
    for s in range(1, e.shape[1]):
        nxt = lse(score[:, :, None] + t[None, :, :] + e[:, s, None, :], axis=1)
        score = np.where(mask[:, s, None], nxt, score)
    log_Z = lse(score, axis=1)
    emit = np.take_along_axis(e, tags[..., None].astype(np.int64), axis=2)[..., 0]
    trans_sc = t[tags[:, :-1].astype(np.int64), tags[:, 1:].astype(np.int64)]
    m = mask[:, 1:].astype(np.float64)
    seq = emit[:, 0] + ((trans_sc + emit[:, 1:]) * m).sum(axis=1)
    return np.float32((log_Z - seq).mean())


def kernel(emissions, transitions, tags, mask):
    emissions = np.asarray(emissions)
    transitions = np.asarray(transitions)
    tags = np.asarray(tags)
    mask = np.asarray(mask)
    if emissions.shape != (B, S, T) or not mask.all():
        return _run_host(emissions, transitions, tags, mask)
    return _run_device(emissions, transitions, tags)
